# revision 19
# baseline (speedup 1.0000x reference)
"""GCN message-passing kernel for 8 Trainium2 NeuronCores (Bass/Tile).

Computes (matching the jax reference):
    h = x @ W_conv                      [N, H]
    node_embed = leaky_relu(D^-1/2 (A+I) D^-1/2 h + b_conv)
    out = sigmoid(leaky(cat(e[i], e[j]) @ W1 + b1) @ W2 + b2)

Only nodes referenced by `index` (the pair head) contribute to the output,
and each core aggregates exactly the nodes its own pair block references
(~3.8K nodes, ~65K edges per core) — no second AllGather is needed; the
pair head reads the core-local embedding table. Sources come from an
AllGather (split in two, overlapped with phase A) of g = (dinv*x) @ W_conv
with dinv folded into x on the host. Edges are gathered in bulk with
dma_gather (pair-packed fp16 rows) and scatter-added on the TensorEngine
via one-hot matmuls; chunks may straddle destination tiles (one matmul per
(chunk, tile) pair) to minimize index padding, since Q7 descriptor
generation is the pacing resource.
"""

import re

import numpy as np

import concourse.bass as bass
import concourse.bacc as bacc
import concourse.mybir as mybir
import concourse.tile as tile
from concourse import library_config
from concourse.bass_utils import run_bass_kernel_spmd

NC = 8
N_NODES = 100000
F_IN = 256
H = 64
NEG = 0.01

P = 128                    # partitions / tile height
TILES = 98                 # full-node tiles per core (phase A)
SHARD = TILES * P          # 12544 nodes per core
HALF = SHARD // 2          # 6272: phase A / AllGather split granularity
NPAD = NC * SHARD          # 100352
HR = NPAD // 4             # 25088: int16-addressable pair rows per range

GROUP_E = 4                # dst tiles per edge gather group
GROUP_P = 16               # pair slot-tiles per gather window
NOMATCH = -999.0


def _node_remap(n):
    """Original node id -> row in the half-split AllGather layout."""
    c, r = n // SHARD, n % SHARD
    h = r // HALF
    return h * (NC * HALF) + c * HALF + (r - h * HALF)


def _wrap_idx(idx):
    """int array [W] (W % 16 == 0) -> [128, W//16] int16 wrapped/replicated."""
    w = idx.reshape(-1, 16).T.astype(np.int16)
    return np.tile(w, (8, 1))


def _sched2(core, tl, loc, bucket, pidx, ntiles, group_sz, nbucket):
    """Multi-tile-chunk gather/scatter schedule, uniform across cores.

    Each item is gathered from pair-packed row `pidx` (bucket selects the
    source window / rhs parity) and scatter-added to column `loc` of tile
    `tl` on core `core`. Rows are laid out per (core, group, bucket) sorted
    by (tile, pidx); chunks are consecutive 128-row windows; a chunk gets
    one matmul per tile present in ANY core's chunk (union), with per-core
    loc planes masking non-members.

    Returns (sched, idx_i16 [NC,128,totidx//16], locmm_f16 [NC,128,n_mm]).
    """
    ngroups = (ntiles + group_sz - 1) // group_sz
    grp = tl // group_sz

    wid = (core * ngroups + grp) * nbucket + bucket
    n_win = NC * ngroups * nbucket
    cnt = np.bincount(wid, minlength=n_win).reshape(NC, ngroups, nbucket)
    K = (cnt + P - 1) // P
    K = K.max(axis=0)                       # [ngroups, nbucket]

    order = np.lexsort((pidx, tl, bucket, grp, core))
    so_core, so_grp, so_b = core[order], grp[order], bucket[order]
    so_tl, so_loc, so_pidx = tl[order], loc[order], pidx[order]
    so_wid = wid[order]
    starts = np.r_[0, np.flatnonzero(np.diff(so_wid)) + 1]
    run_ids = np.zeros(len(so_wid), np.int64)
    run_ids[starts[1:]] = 1
    run_ids = np.cumsum(run_ids)
    rank = np.arange(len(so_wid)) - starts[run_ids]

    win_c0 = np.zeros((ngroups, nbucket), np.int64)
    acc = 0
    for g in range(ngroups):
        for b in range(nbucket):
            win_c0[g, b] = acc
            acc += K[g, b]
    totchunks = acc
    totidx = totchunks * P

    slot = win_c0[so_grp, so_b] * P + rank
    chunk_of = slot // P

    loc_arr = np.full((NC, totidx), -1, np.int64)
    tl_arr = np.full((NC, totidx), -1, np.int64)
    pidx_arr = np.zeros((NC, totidx), np.int64)
    loc_arr[so_core, slot] = so_loc
    tl_arr[so_core, slot] = so_tl
    pidx_arr[so_core, slot] = so_pidx

    pres = set(zip(chunk_of.tolist(), so_tl.tolist()))
    win_of_chunk = np.zeros(totchunks, np.int64)
    for g in range(ngroups):
        for b in range(nbucket):
            win_of_chunk[win_c0[g, b]: win_c0[g, b] + K[g, b]] = g * nbucket + b

    tiles_with_mm = {t for (_, t) in pres}
    for t in range(ntiles):
        if t not in tiles_with_mm:
            g = t // group_sz
            if K[g, 0] == 0:
                raise RuntimeError("empty window for dummy mm")
            pres.add((int(win_c0[g, 0]), t))

    mms = sorted(pres)
    n_mm = len(mms)

    first_mm = {}
    last_mm = {}
    for j, (c, t) in enumerate(mms):
        first_mm.setdefault(t, j)
        last_mm[t] = j

    win_mms = [[] for _ in range(ngroups * nbucket)]
    for j, (c, t) in enumerate(mms):
        win_mms[win_of_chunk[c]].append((j, c, t))

    locmm = np.full((NC, P, n_mm), NOMATCH, np.float16)
    for cc in range(NC):
        la = loc_arr[cc].reshape(totchunks, P)
        ta = tl_arr[cc].reshape(totchunks, P)
        for j, (c, t) in enumerate(mms):
            m = ta[c] == t
            if m.any():
                locmm[cc, m, j] = la[c][m].astype(np.float16)

    idx_i16 = np.zeros((NC, P, totidx // 16), np.int16)
    for cc in range(NC):
        idx_i16[cc] = _wrap_idx(pidx_arr[cc])

    sched = {
        "ngroups": ngroups,
        "nbucket": nbucket,
        "K": K,
        "win_c0": win_c0,
        "win_mms": win_mms,
        "first_mm": first_mm,
        "last_mm": last_mm,
        "n_mm": n_mm,
        "totchunks": totchunks,
        "totidx": totidx,
        "ntiles": ntiles,
        "group_sz": group_sz,
    }
    return sched, idx_i16, locmm


def _prep(inputs):
    x = np.asarray(inputs["x"], np.float32)
    edge_index = np.asarray(inputs["edge_index"], np.int64)
    index = np.asarray(inputs["index"], np.int64)
    W_conv = np.asarray(inputs["W_conv"], np.float32)
    b_conv = np.asarray(inputs["b_conv"], np.float32)
    W1 = np.asarray(inputs["W1"], np.float32)
    b1 = np.asarray(inputs["b1"], np.float32)
    W2 = np.asarray(inputs["W2"], np.float32)
    b2 = np.asarray(inputs["b2"], np.float32)

    n = x.shape[0]
    src = edge_index[0].astype(np.int64)
    dst = edge_index[1].astype(np.int64)

    deg = np.bincount(dst, minlength=NPAD).astype(np.float32) + 1.0
    deg[n:] = 1.0
    dinv = 1.0 / np.sqrt(deg)

    B = index.shape[0]
    PB = B // NC
    assert PB % P == 0
    PCH = PB // P

    # per-core needed node sets (each core aggregates what its pairs read)
    uniq_c = [np.unique(index[c * PB:(c + 1) * PB]) for c in range(NC)]
    NT_E = max(-(-len(u) // P) for u in uniq_c)
    ESHARD = NT_E * P
    slot_of = np.full((NC, n), -1, np.int64)
    for c in range(NC):
        slot_of[c, uniq_c[c]] = np.arange(len(uniq_c[c]))

    # edge stream: an edge goes to every core that needs its dst
    src_new = _node_remap(src)
    uniq_new = [_node_remap(u) for u in uniq_c]
    e_core, e_node, e_slot = [], [], []
    for c in range(NC):
        keep = slot_of[c, dst] >= 0
        e_node.append(src_new[keep])
        e_slot.append(slot_of[c, dst[keep]])
        e_core.append(np.full(keep.sum(), c, np.int64))
        # self-loops
        e_node.append(uniq_new[c])
        e_slot.append(np.arange(len(uniq_c[c]), dtype=np.int64))
        e_core.append(np.full(len(uniq_c[c]), c, np.int64))
    es = np.concatenate(e_node)
    ed = np.concatenate(e_slot)
    ec = np.concatenate(e_core)

    ebucket = 2 * (es >= 2 * HR).astype(np.int64) + (es & 1)
    epidx = (es >> 1) - (ebucket >= 2) * HR
    esched, eidx, elocmm = _sched2(
        core=ec, tl=ed // P, loc=ed % P,
        bucket=ebucket, pidx=epidx, ntiles=NT_E, group_sz=GROUP_E, nbucket=4)

    # pair stream over each core's local e-table (256B-stride rows, so the
    # slot IS the gather index: no parity buckets, zero padding); slots
    # interleaved so pair chunk k reads slot-tiles (2k, 2k+1) -> MLP
    # pipelines with the gather stream
    pair_global = np.arange(B, dtype=np.int64)
    pcore = pair_global // PB
    plocal = pair_global % PB
    s_core = np.concatenate([pcore, pcore])
    ch = plocal // P
    col = plocal % P
    s_slot = np.concatenate([(2 * ch) * P + col, (2 * ch + 1) * P + col])
    s_node = np.concatenate([
        slot_of[pcore, index[:, 0]], slot_of[pcore, index[:, 1]]])
    assert (s_node >= 0).all()
    psched, pidx, plocmm = _sched2(
        core=s_core, tl=s_slot // P, loc=s_slot % P,
        bucket=np.zeros_like(s_node), pidx=s_node,
        ntiles=2 * PCH, group_sz=GROUP_P, nbucket=1)

    # host-folded dinv: g = (dinv * x) @ W
    xpad = np.zeros((NPAD, F_IN), np.float32)
    xpad[:n] = x * dinv[:n, None]
    xT = xpad.T.astype(np.float16)
    xT_shards = [
        np.ascontiguousarray(
            xT[:, c * SHARD:(c + 1) * SHARD].reshape(2, P, SHARD).transpose(1, 0, 2)
        ) for c in range(NC)
    ]
    # dinv over each core's local e-slot space
    dinv_e_sb = []
    for c in range(NC):
        d = np.zeros(ESHARD, np.float32)
        d[:len(uniq_c[c])] = dinv[uniq_c[c]]
        dinv_e_sb.append(np.ascontiguousarray(d.reshape(NT_E, P).T))

    consts = {
        "wc": np.ascontiguousarray(
            W_conv.reshape(2, P, H).transpose(1, 0, 2)).astype(np.float16),
        "bconvb": np.broadcast_to(b_conv, (P, H)).astype(np.float32).copy(),
        "iota": np.broadcast_to(np.arange(P, dtype=np.float16), (P, P)).copy(),
        "ident": np.eye(P, dtype=np.float16),
        "w1": W1.astype(np.float16),
        "b1": b1.reshape(16, 1).astype(np.float32),
        "w2": W2.astype(np.float32),
        "b2t": b2.reshape(1, 1).astype(np.float32),
    }
    sched = {"edge": esched, "pair": psched, "PCH": PCH,
             "NT_E": NT_E, "ESHARD": ESHARD}
    in_maps = []
    for c in range(NC):
        m = {
            "xt": xT_shards[c],
            "dinve": dinv_e_sb[c],
            "elocmm": elocmm[c],
            "egidx": eidx[c],
            "plocmm": plocmm[c],
            "pgidx": pidx[c],
        }
        m.update(consts)
        in_maps.append(m)
    return in_maps, sched


def _emit_scatter2(nc, dt, src_aps, idx_dram, locmm_sb, iota, sched,
                   pools, consume, prefix, post_group=None, idx_eng=None):
    """Gather pair-packed rows per window, build per-matmul one-hot planes,
    matmul-accumulate into per-tile PSUM, hand finished tiles to consume.

    src_aps: bucket -> source AP (pair-packed rows).
    """
    widxp, msgp, ohp, accp = pools
    if idx_eng is None:
        idx_eng = nc.sync
    ngroups = sched["ngroups"]
    nbucket = sched["nbucket"]
    K = sched["K"]
    win_c0 = sched["win_c0"]
    win_mms = sched["win_mms"]
    first_mm = sched["first_mm"]
    last_mm = sched["last_mm"]
    group_sz = sched["group_sz"]
    ntiles = sched["ntiles"]

    acc_tiles = {}

    def _drain(g):
        # consume runs one group late so the DVE queue doesn't head-block
        # on PSUM drains while later windows' one-hots are still pending
        for t in range(g * group_sz, min((g + 1) * group_sz, ntiles)):
            consume(t, acc_tiles.pop(t))
        if post_group is not None:
            post_group(g)

    for g in range(ngroups):
        for b in range(nbucket):
            nch = int(K[g, b])
            if nch == 0:
                continue
            c0 = int(win_c0[g, b])
            nidx = nch * P
            idxt = widxp.tile([P, nidx // 16], dt.int16, tag=f"{prefix}idx{b}",
                              name=f"{prefix}idx_g{g}b{b}")
            idx_eng.dma_start(
                idxt[:], idx_dram[:, c0 * P // 16:(c0 + nch) * P // 16])
            msg = msgp.tile([P, nch, P], dt.float16, tag=f"{prefix}msg{b}",
                            name=f"{prefix}msg_g{g}b{b}")
            nc.gpsimd.dma_gather(
                msg[:], src_aps[b], idxt[:], nidx, nidx, P,
                single_packet=False, queue_num=0)
            mms = win_mms[g * nbucket + b]
            if not mms:
                continue
            j0 = mms[0][0]
            n_mm_w = len(mms)
            oh = ohp.tile([P, n_mm_w, P], dt.float16, tag=f"{prefix}oh{b}",
                          name=f"{prefix}oh_g{g}b{b}")
            nc.vector.tensor_tensor(
                oh[:],
                locmm_sb[:, j0:j0 + n_mm_w].unsqueeze(2).to_broadcast(
                    [P, n_mm_w, P]),
                iota[:, :].unsqueeze(1).to_broadcast([P, n_mm_w, P]),
                mybir.AluOpType.is_equal,
            )
            par = b & 1
            for (j, c, t) in mms:
                if t not in acc_tiles:
                    acc_tiles[t] = accp.tile(
                        [P, H], dt.float32, tag=f"{prefix}acc{t % group_sz}",
                        name=f"{prefix}acc_t{t}")
                nc.tensor.matmul(
                    acc_tiles[t][:],
                    lhsT=oh[:, j - j0, :],
                    rhs=msg[:, c - c0, par * H:(par + 1) * H],
                    start=(j == first_mm[t]),
                    stop=(j == last_mm[t]),
                )
        if g > 0:
            _drain(g - 1)
    _drain(ngroups - 1)


def _build(sched, passes=1):
    dt = mybir.dt
    esched = sched["edge"]
    psched = sched["pair"]
    PCH = sched["PCH"]
    NT_E = sched["NT_E"]
    ESHARD = sched["ESHARD"]

    nc = bacc.Bacc("TRN2", target_bir_lowering=False, debug=False,
                   enable_asserts=False, num_devices=NC, num_swdge_queues=4)

    xt_in = nc.dram_tensor("xt", [P, 2, SHARD], dt.float16, kind="ExternalInput")
    dinve_in = nc.dram_tensor("dinve", [P, NT_E], dt.float32,
                              kind="ExternalInput")
    elocmm_in = nc.dram_tensor("elocmm", [P, esched["n_mm"]], dt.float16,
                               kind="ExternalInput")
    egidx_in = nc.dram_tensor("egidx", [P, esched["totidx"] // 16], dt.int16,
                              kind="ExternalInput")
    plocmm_in = nc.dram_tensor("plocmm", [P, psched["n_mm"]], dt.float16,
                               kind="ExternalInput")
    pgidx_in = nc.dram_tensor("pgidx", [P, psched["totidx"] // 16], dt.int16,
                              kind="ExternalInput")
    wc_in = nc.dram_tensor("wc", [P, 2, H], dt.float16, kind="ExternalInput")
    bconvb_in = nc.dram_tensor("bconvb", [P, H], dt.float32, kind="ExternalInput")
    iota_in = nc.dram_tensor("iota", [P, P], dt.float16, kind="ExternalInput")
    ident_in = nc.dram_tensor("ident", [P, P], dt.float16, kind="ExternalInput")
    w1_in = nc.dram_tensor("w1", [P, 16], dt.float16, kind="ExternalInput")
    b1_in = nc.dram_tensor("b1", [16, 1], dt.float32, kind="ExternalInput")
    w2_in = nc.dram_tensor("w2", [16, 1], dt.float32, kind="ExternalInput")
    b2_in = nc.dram_tensor("b2t", [1, 1], dt.float32, kind="ExternalInput")
    outp = nc.dram_tensor("out", [PCH * P, 1], dt.float32, kind="ExternalOutput")

    g_shard = nc.dram_tensor("g_shard", [SHARD, H], dt.float16)
    g_full = nc.dram_tensor("g_full", [NPAD, H], dt.float16, addr_space="Shared")
    # local e-table with 256B row stride; only the first H columns are
    # written/read (the tail pads rows to dma_gather's 256B granularity)
    e_tab = nc.dram_tensor("e_tab", [ESHARD, 2 * H], dt.float16)

    g_pairs = g_full[:, :].rearrange("(r two) f -> r (two f)", two=2)

    with tile.TileContext(nc) as tc:
        nc.gpsimd.load_library(library_config.mlp)

        with (
            tc.tile_pool(name="const", bufs=1) as cpool,
            tc.tile_pool(name="dinvp", bufs=1) as dpool,
        ):
            wc_sb = cpool.tile([P, 2, H], dt.float16)
            nc.sync.dma_start(wc_sb[:], wc_in[:, :, :])
            bconvb = cpool.tile([P, H], dt.float32)
            nc.sync.dma_start(bconvb[:], bconvb_in[:, :])
            iota = cpool.tile([P, P], dt.float16)
            nc.sync.dma_start(iota[:], iota_in[:, :])
            ident = cpool.tile([P, P], dt.float16)
            nc.sync.dma_start(ident[:], ident_in[:, :])
            w1_sb = cpool.tile([P, 16], dt.float16)
            nc.sync.dma_start(w1_sb[:], w1_in[:, :])
            b1_sb = cpool.tile([16, 1], dt.float32)
            nc.sync.dma_start(b1_sb[:], b1_in[:, :])
            w2_sb = cpool.tile([16, 1], dt.float32)
            nc.sync.dma_start(w2_sb[:], w2_in[:, :])
            b2_sb = cpool.tile([1, 1], dt.float32)
            nc.sync.dma_start(b2_sb[:], b2_in[:, :])
            elocmm_sb = cpool.tile([P, esched["n_mm"]], dt.float16)
            nc.sync.dma_start(elocmm_sb[:], elocmm_in[:, :])

            dinve = dpool.tile([P, NT_E], dt.float32)
            nc.sync.dma_start(dinve[:], dinve_in[:, :])

            def _one_pass():
                # ------- phase A: g = x' @ W, AllGather split in halves -------
                XBLK = 7
                HTILES = HALF // P        # tiles 0..48 cover rows [0, HALF)
                assert HTILES % XBLK == 0
                with (
                    tc.tile_pool(name="xtp", bufs=2) as xtp,
                    tc.tile_pool(name="hps", bufs=4, space="PSUM") as hps,
                    tc.tile_pool(name="gsb", bufs=1) as gsbp,
                ):
                    g_sb = gsbp.tile([P, TILES, H], dt.float16)
                    for blk in range((TILES + XBLK - 1) // XBLK):
                        t0, t1 = blk * XBLK, min((blk + 1) * XBLK, TILES)
                        xt_sb = xtp.tile([P, 2, (t1 - t0) * P], dt.float16, tag="xt")
                        nc.sync.dma_start(xt_sb[:], xt_in[:, :, t0 * P: t1 * P])
                        for t in range(t0, t1):
                            h_ps = hps.tile([P, H], dt.float32)
                            for k in range(2):
                                nc.tensor.matmul(
                                    h_ps[:],
                                    lhsT=xt_sb[:, k, (t - t0) * P:(t - t0 + 1) * P],
                                    rhs=wc_sb[:, k, :],
                                    start=(k == 0), stop=(k == 1),
                                )
                            nc.scalar.activation(
                                g_sb[:, t, :], h_ps[:],
                                mybir.ActivationFunctionType.Copy,
                                bias=0.0, scale=1.0)
                        if t1 == HTILES:
                            # first half ready: store + gather while 2nd half runs
                            nc.sync.dma_start(
                                g_shard[0:HALF, :].rearrange(
                                    "(t p) f -> p t f", p=P),
                                g_sb[:, 0:HALF // P, :],
                            )
                            nc.gpsimd.collective_compute(
                                "AllGather", mybir.AluOpType.bypass,
                                replica_groups=[list(range(NC))],
                                ins=[g_shard[0:HALF, :].opt()],
                                outs=[g_full[0:NC * HALF, :].opt()],
                            )
                    nc.sync.dma_start(
                        g_shard[HALF:, :].rearrange("(t p) f -> p t f", p=P),
                        g_sb[:, HALF // P:, :],
                    )

                nc.gpsimd.collective_compute(
                    "AllGather", mybir.AluOpType.bypass,
                    replica_groups=[list(range(NC))],
                    ins=[g_shard[HALF:, :].opt()],
                    outs=[g_full[NC * HALF:, :].opt()],
                )

                # ---------------- phase C: aggregate per dst tile ----------------
                with (
                    tc.tile_pool(name="ewidx", bufs=3) as widxp,
                    tc.tile_pool(name="emsg", bufs=3) as msgp,
                    tc.tile_pool(name="eoh", bufs=3) as ohp,
                    tc.tile_pool(name="eacc", bufs=2, space="PSUM") as accp,
                    tc.tile_pool(name="epost", bufs=4) as postp,
                    tc.tile_pool(name="eemb", bufs=4) as embp,
                ):
                    def consume_edge(t, a):
                        e1 = postp.tile([P, H], dt.float32, tag="e1", name=f"e1_{t}")
                        nc.vector.tensor_scalar(
                            e1[:], a[:], dinve[:, t:t + 1], None,
                            mybir.AluOpType.mult)
                        nc.vector.tensor_tensor(
                            e1[:], e1[:], bconvb[:], mybir.AluOpType.add)
                        m = postp.tile([P, H], dt.float32, tag="m", name=f"m_{t}")
                        nc.scalar.activation(
                            m[:], e1[:], mybir.ActivationFunctionType.Copy,
                            bias=0.0, scale=NEG)
                        emb = embp.tile([P, H], dt.float16, name=f"emb_{t}")
                        nc.vector.tensor_tensor(
                            emb[:], e1[:], m[:], mybir.AluOpType.max)
                        nc.sync.dma_start(e_tab[t * P:(t + 1) * P, 0:H], emb[:])

                    e_src_aps = [g_pairs, g_pairs,
                                 g_pairs[HR:, :], g_pairs[HR:, :]]
                    _emit_scatter2(nc, dt, e_src_aps, egidx_in, elocmm_sb, iota,
                                   esched, (widxp, msgp, ohp, accp),
                                   consume_edge, "e")

                # ------- phase D: pair gather + permute-matmul + MLP -------
                # pair slots are laid out so gather chunk == slot tile and
                # tiles (2k, 2k+1) hold pair-chunk k's xi/xj lookups; one
                # matmul msg.T @ onehot produces each 64-row half of xijt
                # (feature-major, pair-ordered) straight from the gather.
                with (
                    tc.tile_pool(name="pconst", bufs=1) as pcpool,
                    tc.tile_pool(name="pwidx", bufs=2) as pwidxp,
                    tc.tile_pool(name="pmsg", bufs=2) as pmsgp,
                    tc.tile_pool(name="poh", bufs=2) as pohp,
                    tc.tile_pool(name="ptps", bufs=3, space="PSUM") as ptps,
                    tc.tile_pool(name="pzps", bufs=2, space="PSUM") as pzps,
                    tc.tile_pool(name="pops", bufs=2, space="PSUM") as pops,
                    tc.tile_pool(name="psb", bufs=4) as psbp,
                ):
                    plocmm_sb = pcpool.tile([P, psched["n_mm"]], dt.float16)
                    nc.scalar.dma_start(plocmm_sb[:], plocmm_in[:, :])

                    ngroups_p = psched["ngroups"]
                    Kp = psched["K"]
                    win_c0_p = psched["win_c0"]
                    msg_w = {}
                    oh_w = {}
                    for g in range(ngroups_p):
                        nch = int(Kp[g, 0])
                        c0 = int(win_c0_p[g, 0])
                        nidx = nch * P
                        idxt = pwidxp.tile([P, nidx // 16], dt.int16,
                                           tag="pidx", name=f"pidx_g{g}")
                        nc.scalar.dma_start(
                            idxt[:],
                            pgidx_in[:, c0 * P // 16:(c0 + nch) * P // 16])
                        msg = pmsgp.tile([P, nch, P], dt.float16, tag="pmsg",
                                         name=f"pmsg_g{g}")
                        nc.gpsimd.dma_gather(
                            msg[:], e_tab[:, :], idxt[:], nidx, nidx, P,
                            single_packet=False, queue_num=0)
                        oh = pohp.tile([P, nch, P], dt.float16, tag="poh",
                                       name=f"poh_g{g}")
                        nc.vector.tensor_tensor(
                            oh[:],
                            plocmm_sb[:, c0:c0 + nch].unsqueeze(2).to_broadcast(
                                [P, nch, P]),
                            iota[:, :].unsqueeze(1).to_broadcast([P, nch, P]),
                            mybir.AluOpType.is_equal,
                        )
                        msg_w[g] = (msg, c0)
                        oh_w[g] = oh

                    for k in range(PCH):
                        xt_ps = ptps.tile([P, P], dt.float32)
                        for half in range(2):
                            t = 2 * k + half
                            g = t // GROUP_P
                            msg, c0 = msg_w[g]
                            nc.tensor.matmul(
                                xt_ps[half * H:(half + 1) * H, :],
                                lhsT=msg[:, t - c0, 0:H],
                                rhs=oh_w[g][:, t - c0, :],
                                start=True, stop=True,
                            )
                        xijt = psbp.tile([P, P], dt.float16, tag="xijt")
                        nc.vector.tensor_copy(xijt[:], xt_ps[:])
                        z_ps = pzps.tile([16, P], dt.float32)
                        nc.tensor.matmul(z_ps[:], lhsT=w1_sb[:], rhs=xijt[:],
                                         start=True, stop=True)
                        zb = psbp.tile([16, P], dt.float32, tag="zb")
                        nc.vector.tensor_scalar(
                            zb[:], z_ps[:], b1_sb[:, 0:1], None,
                            mybir.AluOpType.add)
                        m2 = psbp.tile([16, P], dt.float32, tag="m2")
                        nc.scalar.activation(
                            m2[:], zb[:], mybir.ActivationFunctionType.Copy,
                            bias=0.0, scale=NEG)
                        z2 = psbp.tile([16, P], dt.float32, tag="z2")
                        nc.vector.tensor_tensor(z2[:], zb[:], m2[:],
                                                mybir.AluOpType.max)
                        o_ps = pops.tile([1, P], dt.float32)
                        nc.tensor.matmul(o_ps[:], lhsT=w2_sb[:], rhs=z2[:],
                                         start=True, stop=True)
                        osb = psbp.tile([1, P], dt.float32, tag="osb")
                        nc.scalar.activation(
                            osb[:], o_ps[:], mybir.ActivationFunctionType.Sigmoid,
                            bias=b2_sb[:, 0:1], scale=1.0)
                        nc.sync.dma_start(
                            outp[k * P:(k + 1) * P, :].rearrange("r one -> one r"),
                            osb[0:1, :])

            for _ in range(passes):
                _one_pass()

    # align each gather's SWDGE queue with its Tile-assigned DMA lane so
    # semaphore<->queue locking stays consistent (4-way parallel desc gen)
    for blk in nc.m.functions[0].blocks:
        for inst in blk.instructions:
            if isinstance(inst, mybir.InstDMAGatherAnt):
                si = inst.sync_info
                for u in (si.on_update if si else []):
                    mm = re.match(r"DMASW(\d+)_", u.ant_name or "")
                    if mm:
                        inst.queue_num = int(mm.group(1)) % 4
                        break

    nc.compile()
    return nc


def kernel(**inputs) -> np.ndarray:
    in_maps, sched = _prep(inputs)
    nc = _build(sched)
    res = run_bass_kernel_spmd(nc, in_maps, list(range(NC)))
    out = np.concatenate([res.results[c]["out"] for c in range(NC)], axis=0)
    return out.astype(np.float32)


# revision 26
# speedup vs baseline: 1.0689x; 1.0689x over previous
"""GCN message-passing kernel for 8 Trainium2 NeuronCores (Bass/Tile).

Computes (matching the jax reference):
    h = x @ W_conv                      [N, H]
    node_embed = leaky_relu(D^-1/2 (A+I) D^-1/2 h + b_conv)
    out = sigmoid(leaky(cat(e[i], e[j]) @ W1 + b1) @ W2 + b2)

Only nodes referenced by `index` (the pair head) contribute to the output,
and each core aggregates exactly the nodes its own pair block references
(~3.8K nodes, ~65K edges per core) — no second AllGather is needed; the
pair head reads the core-local embedding table. Sources come from an
AllGather (split in two, overlapped with phase A) of g = (dinv*x) @ W_conv
with dinv folded into x on the host. Edges are gathered in bulk with
dma_gather (pair-packed fp16 rows) and scatter-added on the TensorEngine
via one-hot matmuls; chunks may straddle destination tiles (one matmul per
(chunk, tile) pair) to minimize index padding, since Q7 descriptor
generation is the pacing resource.
"""

import re

import numpy as np

import concourse.bass as bass
import concourse.bacc as bacc
import concourse.mybir as mybir
import concourse.tile as tile
from concourse import library_config
from concourse.bass_utils import run_bass_kernel_spmd

NC = 8
N_NODES = 100000
F_IN = 256
H = 64
NEG = 0.01

P = 128                    # partitions / tile height
TILES = 98                 # full-node tiles per core (phase A)
SHARD = TILES * P          # 12544 nodes per core
HALF = SHARD // 2          # 6272: phase A / AllGather split granularity
NPAD = NC * SHARD          # 100352
HR = NPAD // 4             # 25088: int16-addressable pair rows per range

GROUP_E = 4                # dst tiles per edge gather group
GROUP_P = 16               # pair slot-tiles per gather window
NOMATCH = -999.0


def _node_remap(n):
    """Original node id -> row in the half-split AllGather layout."""
    c, r = n // SHARD, n % SHARD
    h = r // HALF
    return h * (NC * HALF) + c * HALF + (r - h * HALF)


def _wrap_idx(idx):
    """int array [W] (W % 16 == 0) -> [128, W//16] int16 wrapped/replicated."""
    w = idx.reshape(-1, 16).T.astype(np.int16)
    return np.tile(w, (8, 1))


def _sched2(core, tl, loc, bucket, pidx, ntiles, group_sz, nbucket):
    """Multi-tile-chunk gather/scatter schedule, uniform across cores.

    Each item is gathered from pair-packed row `pidx` (bucket selects the
    source window / rhs parity) and scatter-added to column `loc` of tile
    `tl` on core `core`. Rows are laid out per (core, group, bucket) sorted
    by (tile, pidx); chunks are consecutive 128-row windows; a chunk gets
    one matmul per tile present in ANY core's chunk (union), with per-core
    loc planes masking non-members.

    Returns (sched, idx_i16 [NC,128,totidx//16], locmm_f16 [NC,128,n_mm]).
    """
    ngroups = (ntiles + group_sz - 1) // group_sz
    grp = tl // group_sz

    wid = (core * ngroups + grp) * nbucket + bucket
    n_win = NC * ngroups * nbucket
    cnt = np.bincount(wid, minlength=n_win).reshape(NC, ngroups, nbucket)
    K = (cnt + P - 1) // P
    K = K.max(axis=0)                       # [ngroups, nbucket]

    order = np.lexsort((pidx, tl, bucket, grp, core))
    so_core, so_grp, so_b = core[order], grp[order], bucket[order]
    so_tl, so_loc, so_pidx = tl[order], loc[order], pidx[order]
    so_wid = wid[order]
    starts = np.r_[0, np.flatnonzero(np.diff(so_wid)) + 1]
    run_ids = np.zeros(len(so_wid), np.int64)
    run_ids[starts[1:]] = 1
    run_ids = np.cumsum(run_ids)
    rank = np.arange(len(so_wid)) - starts[run_ids]

    win_c0 = np.zeros((ngroups, nbucket), np.int64)
    acc = 0
    for g in range(ngroups):
        for b in range(nbucket):
            win_c0[g, b] = acc
            acc += K[g, b]
    totchunks = acc
    totidx = totchunks * P

    slot = win_c0[so_grp, so_b] * P + rank
    chunk_of = slot // P

    loc_arr = np.full((NC, totidx), -1, np.int64)
    tl_arr = np.full((NC, totidx), -1, np.int64)
    pidx_arr = np.zeros((NC, totidx), np.int64)
    loc_arr[so_core, slot] = so_loc
    tl_arr[so_core, slot] = so_tl
    pidx_arr[so_core, slot] = so_pidx

    pres = set(zip(chunk_of.tolist(), so_tl.tolist()))
    win_of_chunk = np.zeros(totchunks, np.int64)
    for g in range(ngroups):
        for b in range(nbucket):
            win_of_chunk[win_c0[g, b]: win_c0[g, b] + K[g, b]] = g * nbucket + b

    tiles_with_mm = {t for (_, t) in pres}
    for t in range(ntiles):
        if t not in tiles_with_mm:
            g = t // group_sz
            if K[g, 0] == 0:
                raise RuntimeError("empty window for dummy mm")
            pres.add((int(win_c0[g, 0]), t))

    mms = sorted(pres)
    n_mm = len(mms)

    first_mm = {}
    last_mm = {}
    for j, (c, t) in enumerate(mms):
        first_mm.setdefault(t, j)
        last_mm[t] = j

    win_mms = [[] for _ in range(ngroups * nbucket)]
    for j, (c, t) in enumerate(mms):
        win_mms[win_of_chunk[c]].append((j, c, t))

    locmm = np.full((NC, P, n_mm), NOMATCH, np.float16)
    for cc in range(NC):
        la = loc_arr[cc].reshape(totchunks, P)
        ta = tl_arr[cc].reshape(totchunks, P)
        for j, (c, t) in enumerate(mms):
            m = ta[c] == t
            if m.any():
                locmm[cc, m, j] = la[c][m].astype(np.float16)

    idx_i16 = np.zeros((NC, P, totidx // 16), np.int16)
    for cc in range(NC):
        idx_i16[cc] = _wrap_idx(pidx_arr[cc])

    sched = {
        "ngroups": ngroups,
        "nbucket": nbucket,
        "K": K,
        "win_c0": win_c0,
        "win_mms": win_mms,
        "first_mm": first_mm,
        "last_mm": last_mm,
        "n_mm": n_mm,
        "totchunks": totchunks,
        "totidx": totidx,
        "ntiles": ntiles,
        "group_sz": group_sz,
    }
    return sched, idx_i16, locmm


def _prep(inputs):
    x = np.asarray(inputs["x"], np.float32)
    edge_index = np.asarray(inputs["edge_index"], np.int64)
    index = np.asarray(inputs["index"], np.int64)
    W_conv = np.asarray(inputs["W_conv"], np.float32)
    b_conv = np.asarray(inputs["b_conv"], np.float32)
    W1 = np.asarray(inputs["W1"], np.float32)
    b1 = np.asarray(inputs["b1"], np.float32)
    W2 = np.asarray(inputs["W2"], np.float32)
    b2 = np.asarray(inputs["b2"], np.float32)

    n = x.shape[0]
    src = edge_index[0].astype(np.int64)
    dst = edge_index[1].astype(np.int64)

    deg = np.bincount(dst, minlength=NPAD).astype(np.float32) + 1.0
    deg[n:] = 1.0
    dinv = 1.0 / np.sqrt(deg)

    B = index.shape[0]
    PB = B // NC
    assert PB % P == 0
    PCH = PB // P

    # per-core needed node sets (each core aggregates what its pairs read)
    uniq_c = [np.unique(index[c * PB:(c + 1) * PB]) for c in range(NC)]
    NT_E = max(-(-len(u) // P) for u in uniq_c)
    ESHARD = NT_E * P
    slot_of = np.full((NC, n), -1, np.int64)
    for c in range(NC):
        slot_of[c, uniq_c[c]] = np.arange(len(uniq_c[c]))

    # edge stream: an edge goes to every core that needs its dst
    src_new = _node_remap(src)
    uniq_new = [_node_remap(u) for u in uniq_c]
    e_core, e_node, e_slot = [], [], []
    for c in range(NC):
        keep = slot_of[c, dst] >= 0
        e_node.append(src_new[keep])
        e_slot.append(slot_of[c, dst[keep]])
        e_core.append(np.full(keep.sum(), c, np.int64))
        # self-loops
        e_node.append(uniq_new[c])
        e_slot.append(np.arange(len(uniq_c[c]), dtype=np.int64))
        e_core.append(np.full(len(uniq_c[c]), c, np.int64))
    es = np.concatenate(e_node)
    ed = np.concatenate(e_slot)
    ec = np.concatenate(e_core)

    ebucket = 2 * (es >= 2 * HR).astype(np.int64) + (es & 1)
    epidx = (es >> 1) - (ebucket >= 2) * HR
    esched, eidx, elocmm = _sched2(
        core=ec, tl=ed // P, loc=ed % P,
        bucket=ebucket, pidx=epidx, ntiles=NT_E, group_sz=GROUP_E, nbucket=4)

    # pair stream over each core's local e-table (256B-stride rows, so the
    # slot IS the gather index: no parity buckets, zero padding); slots
    # interleaved so pair chunk k reads slot-tiles (2k, 2k+1) -> MLP
    # pipelines with the gather stream
    pair_global = np.arange(B, dtype=np.int64)
    pcore = pair_global // PB
    plocal = pair_global % PB
    s_core = np.concatenate([pcore, pcore])
    ch = plocal // P
    col = plocal % P
    s_slot = np.concatenate([(2 * ch) * P + col, (2 * ch + 1) * P + col])
    s_node = np.concatenate([
        slot_of[pcore, index[:, 0]], slot_of[pcore, index[:, 1]]])
    assert (s_node >= 0).all()
    psched, pidx, plocmm = _sched2(
        core=s_core, tl=s_slot // P, loc=s_slot % P,
        bucket=np.zeros_like(s_node), pidx=s_node,
        ntiles=2 * PCH, group_sz=GROUP_P, nbucket=1)

    # host-folded dinv: g = (dinv * x) @ W
    xpad = np.zeros((NPAD, F_IN), np.float32)
    xpad[:n] = x * dinv[:n, None]
    xT = xpad.T.astype(np.float16)
    xT_shards = [
        np.ascontiguousarray(
            xT[:, c * SHARD:(c + 1) * SHARD].reshape(2, P, SHARD).transpose(1, 0, 2)
        ) for c in range(NC)
    ]
    # dinv over each core's local e-slot space, and bconv/dinv bias tables
    # (added into the PSUM accumulator via an identity matmul so the whole
    # consume is one Lrelu activation)
    dinv_e_sb = []
    bde_sb = []
    for c in range(NC):
        d = np.zeros(ESHARD, np.float32)
        d[:len(uniq_c[c])] = dinv[uniq_c[c]]
        dinv_e_sb.append(np.ascontiguousarray(d.reshape(NT_E, P).T))
        bde = np.zeros((ESHARD, H), np.float32)
        nu = len(uniq_c[c])
        bde[:nu] = b_conv[None, :] / d[:nu, None]
        bde_sb.append(np.ascontiguousarray(
            bde.reshape(NT_E, P, H).transpose(1, 0, 2)).astype(np.float16))

    consts = {
        "wc": np.ascontiguousarray(
            W_conv.reshape(2, P, H).transpose(1, 0, 2)).astype(np.float16),
        "iota": np.broadcast_to(np.arange(P, dtype=np.float16), (P, P)).copy(),
        "ident": np.eye(P, dtype=np.float16),
        "w1": W1.astype(np.float16),
        "b1": b1.reshape(16, 1).astype(np.float32),
        "w2": W2.astype(np.float32),
        "b2t": b2.reshape(1, 1).astype(np.float32),
    }
    sched = {"edge": esched, "pair": psched, "PCH": PCH,
             "NT_E": NT_E, "ESHARD": ESHARD}
    in_maps = []
    for c in range(NC):
        m = {
            "xt": xT_shards[c],
            "dinve": dinv_e_sb[c],
            "bde": bde_sb[c],
            "elocmm": elocmm[c],
            "egidx": eidx[c],
            "plocmm": plocmm[c],
            "pgidx": pidx[c],
        }
        m.update(consts)
        in_maps.append(m)
    return in_maps, sched


def _emit_scatter2(nc, dt, src_aps, idx_dram, locmm_sb, iota, sched,
                   pools, consume, prefix, post_group=None, idx_eng=None,
                   acc_init=None):
    """Gather pair-packed rows per window, build per-matmul one-hot planes,
    matmul-accumulate into per-tile PSUM, hand finished tiles to consume.

    src_aps: bucket -> source AP (pair-packed rows).
    """
    widxp, msgp, ohp, accp = pools
    if idx_eng is None:
        idx_eng = nc.sync
    ngroups = sched["ngroups"]
    nbucket = sched["nbucket"]
    K = sched["K"]
    win_c0 = sched["win_c0"]
    win_mms = sched["win_mms"]
    first_mm = sched["first_mm"]
    last_mm = sched["last_mm"]
    group_sz = sched["group_sz"]
    ntiles = sched["ntiles"]

    acc_tiles = {}

    def _drain(g):
        # consume runs one group late so the DVE queue doesn't head-block
        # on PSUM drains while later windows' one-hots are still pending
        for t in range(g * group_sz, min((g + 1) * group_sz, ntiles)):
            consume(t, acc_tiles.pop(t))
        if post_group is not None:
            post_group(g)

    for g in range(ngroups):
        for b in range(nbucket):
            nch = int(K[g, b])
            if nch == 0:
                continue
            c0 = int(win_c0[g, b])
            nidx = nch * P
            idxt = widxp.tile([P, nidx // 16], dt.int16, tag=f"{prefix}idx{b}",
                              name=f"{prefix}idx_g{g}b{b}")
            idx_eng.dma_start(
                idxt[:], idx_dram[:, c0 * P // 16:(c0 + nch) * P // 16])
            msg = msgp.tile([P, nch, P], dt.float16, tag=f"{prefix}msg{b}",
                            name=f"{prefix}msg_g{g}b{b}")
            nc.gpsimd.dma_gather(
                msg[:], src_aps[b], idxt[:], nidx, nidx, P,
                single_packet=False, queue_num=0)
            mms = win_mms[g * nbucket + b]
            if not mms:
                continue
            j0 = mms[0][0]
            n_mm_w = len(mms)
            oh = ohp.tile([P, n_mm_w, P], dt.float16, tag=f"{prefix}oh{b}",
                          name=f"{prefix}oh_g{g}b{b}")
            nc.vector.tensor_tensor(
                oh[:],
                locmm_sb[:, j0:j0 + n_mm_w].unsqueeze(2).to_broadcast(
                    [P, n_mm_w, P]),
                iota[:, :].unsqueeze(1).to_broadcast([P, n_mm_w, P]),
                mybir.AluOpType.is_equal,
            )
            par = b & 1
            for (j, c, t) in mms:
                if t not in acc_tiles:
                    acc_tiles[t] = accp.tile(
                        [P, H], dt.float32, tag=f"{prefix}acc{t % group_sz}",
                        name=f"{prefix}acc_t{t}")
                    if acc_init is not None:
                        acc_init(t, acc_tiles[t])
                nc.tensor.matmul(
                    acc_tiles[t][:],
                    lhsT=oh[:, j - j0, :],
                    rhs=msg[:, c - c0, par * H:(par + 1) * H],
                    start=(acc_init is None and j == first_mm[t]),
                    stop=(j == last_mm[t]),
                )
        if g > 0:
            _drain(g - 1)
    _drain(ngroups - 1)


def _build(sched, passes=1):
    dt = mybir.dt
    esched = sched["edge"]
    psched = sched["pair"]
    PCH = sched["PCH"]
    NT_E = sched["NT_E"]
    ESHARD = sched["ESHARD"]

    nc = bacc.Bacc("TRN2", target_bir_lowering=False, debug=False,
                   enable_asserts=False, num_devices=NC, num_swdge_queues=4)

    xt_in = nc.dram_tensor("xt", [P, 2, SHARD], dt.float16, kind="ExternalInput")
    dinve_in = nc.dram_tensor("dinve", [P, NT_E], dt.float32,
                              kind="ExternalInput")
    bde_in = nc.dram_tensor("bde", [P, NT_E, H], dt.float16,
                            kind="ExternalInput")
    elocmm_in = nc.dram_tensor("elocmm", [P, esched["n_mm"]], dt.float16,
                               kind="ExternalInput")
    egidx_in = nc.dram_tensor("egidx", [P, esched["totidx"] // 16], dt.int16,
                              kind="ExternalInput")
    plocmm_in = nc.dram_tensor("plocmm", [P, psched["n_mm"]], dt.float16,
                               kind="ExternalInput")
    pgidx_in = nc.dram_tensor("pgidx", [P, psched["totidx"] // 16], dt.int16,
                              kind="ExternalInput")
    wc_in = nc.dram_tensor("wc", [P, 2, H], dt.float16, kind="ExternalInput")
    iota_in = nc.dram_tensor("iota", [P, P], dt.float16, kind="ExternalInput")
    ident_in = nc.dram_tensor("ident", [P, P], dt.float16, kind="ExternalInput")
    w1_in = nc.dram_tensor("w1", [P, 16], dt.float16, kind="ExternalInput")
    b1_in = nc.dram_tensor("b1", [16, 1], dt.float32, kind="ExternalInput")
    w2_in = nc.dram_tensor("w2", [16, 1], dt.float32, kind="ExternalInput")
    b2_in = nc.dram_tensor("b2t", [1, 1], dt.float32, kind="ExternalInput")
    outp = nc.dram_tensor("out", [PCH * P, 1], dt.float32, kind="ExternalOutput")

    g_shard = nc.dram_tensor("g_shard", [SHARD, H], dt.float16)
    g_full = nc.dram_tensor("g_full", [NPAD, H], dt.float16, addr_space="Shared")
    # local e-table with 256B row stride; only the first H columns are
    # written/read (the tail pads rows to dma_gather's 256B granularity)
    e_tab = nc.dram_tensor("e_tab", [ESHARD, 2 * H], dt.float16)

    g_pairs = g_full[:, :].rearrange("(r two) f -> r (two f)", two=2)

    with tile.TileContext(nc) as tc:
        nc.gpsimd.load_library(library_config.mlp)

        with (
            tc.tile_pool(name="const", bufs=1) as cpool,
            tc.tile_pool(name="dinvp", bufs=1) as dpool,
        ):
            wc_sb = cpool.tile([P, 2, H], dt.float16)
            nc.sync.dma_start(wc_sb[:], wc_in[:, :, :])
            iota = cpool.tile([P, P], dt.float16)
            nc.sync.dma_start(iota[:], iota_in[:, :])
            ident = cpool.tile([P, P], dt.float16)
            nc.sync.dma_start(ident[:], ident_in[:, :])
            w1_sb = cpool.tile([P, 16], dt.float16)
            nc.sync.dma_start(w1_sb[:], w1_in[:, :])
            b1_sb = cpool.tile([16, 1], dt.float32)
            nc.sync.dma_start(b1_sb[:], b1_in[:, :])
            w2_sb = cpool.tile([16, 1], dt.float32)
            nc.sync.dma_start(w2_sb[:], w2_in[:, :])
            b2_sb = cpool.tile([1, 1], dt.float32)
            nc.sync.dma_start(b2_sb[:], b2_in[:, :])
            elocmm_sb = cpool.tile([P, esched["n_mm"]], dt.float16)
            nc.sync.dma_start(elocmm_sb[:], elocmm_in[:, :])

            dinve = dpool.tile([P, NT_E], dt.float32)
            nc.sync.dma_start(dinve[:], dinve_in[:, :])
            bde_sb = dpool.tile([P, NT_E, H], dt.float16)
            nc.sync.dma_start(bde_sb[:], bde_in[:, :, :])

            def _one_pass():
                # ------- phase A: g = x' @ W, AllGather split in halves -------
                XBLK = 7
                HTILES = HALF // P        # tiles 0..48 cover rows [0, HALF)
                assert HTILES % XBLK == 0
                with (
                    tc.tile_pool(name="xtp", bufs=2) as xtp,
                    tc.tile_pool(name="hps", bufs=4, space="PSUM") as hps,
                    tc.tile_pool(name="gsb", bufs=1) as gsbp,
                ):
                    g_sb = gsbp.tile([P, TILES, H], dt.float16)
                    for blk in range((TILES + XBLK - 1) // XBLK):
                        t0, t1 = blk * XBLK, min((blk + 1) * XBLK, TILES)
                        xt_sb = xtp.tile([P, 2, (t1 - t0) * P], dt.float16, tag="xt")
                        nc.sync.dma_start(xt_sb[:], xt_in[:, :, t0 * P: t1 * P])
                        for t in range(t0, t1):
                            h_ps = hps.tile([P, H], dt.float32)
                            for k in range(2):
                                nc.tensor.matmul(
                                    h_ps[:],
                                    lhsT=xt_sb[:, k, (t - t0) * P:(t - t0 + 1) * P],
                                    rhs=wc_sb[:, k, :],
                                    start=(k == 0), stop=(k == 1),
                                )
                            nc.scalar.activation(
                                g_sb[:, t, :], h_ps[:],
                                mybir.ActivationFunctionType.Copy,
                                bias=0.0, scale=1.0)
                        if t1 == HTILES:
                            # first half ready: store + gather while 2nd half runs
                            nc.sync.dma_start(
                                g_shard[0:HALF, :].rearrange(
                                    "(t p) f -> p t f", p=P),
                                g_sb[:, 0:HALF // P, :],
                            )
                            nc.gpsimd.collective_compute(
                                "AllGather", mybir.AluOpType.bypass,
                                replica_groups=[list(range(NC))],
                                ins=[g_shard[0:HALF, :].opt()],
                                outs=[g_full[0:NC * HALF, :].opt()],
                            )
                    nc.sync.dma_start(
                        g_shard[HALF:, :].rearrange("(t p) f -> p t f", p=P),
                        g_sb[:, HALF // P:, :],
                    )

                nc.gpsimd.collective_compute(
                    "AllGather", mybir.AluOpType.bypass,
                    replica_groups=[list(range(NC))],
                    ins=[g_shard[HALF:, :].opt()],
                    outs=[g_full[NC * HALF:, :].opt()],
                )

                # ---------------- phase C: aggregate per dst tile ----------------
                with (
                    tc.tile_pool(name="ewidx", bufs=3) as widxp,
                    tc.tile_pool(name="emsg", bufs=3) as msgp,
                    tc.tile_pool(name="eoh", bufs=3) as ohp,
                    tc.tile_pool(name="eacc", bufs=2, space="PSUM") as accp,
                    tc.tile_pool(name="eemb", bufs=4) as embp,
                ):
                    def acc_init_edge(t, a):
                        # seed PSUM with bconv/dinv so consume is one Lrelu
                        nc.tensor.matmul(a[:], lhsT=ident[:],
                                         rhs=bde_sb[:, t, :],
                                         start=True, stop=False)

                    def consume_edge(t, a):
                        emb = embp.tile([P, H], dt.float16, name=f"emb_{t}")
                        nc.scalar.activation(
                            emb[:], a[:], mybir.ActivationFunctionType.Lrelu,
                            bias=0.0, scale=dinve[:, t:t + 1], alpha=NEG)
                        nc.sync.dma_start(e_tab[t * P:(t + 1) * P, 0:H], emb[:])

                    e_src_aps = [g_pairs, g_pairs,
                                 g_pairs[HR:, :], g_pairs[HR:, :]]
                    _emit_scatter2(nc, dt, e_src_aps, egidx_in, elocmm_sb, iota,
                                   esched, (widxp, msgp, ohp, accp),
                                   consume_edge, "e", acc_init=acc_init_edge)

                # ------- phase D: pair gather + permute-matmul + MLP -------
                # pair slots are laid out so gather chunk == slot tile and
                # tiles (2k, 2k+1) hold pair-chunk k's xi/xj lookups; one
                # matmul msg.T @ onehot produces each 64-row half of xijt
                # (feature-major, pair-ordered) straight from the gather.
                with (
                    tc.tile_pool(name="pconst", bufs=1) as pcpool,
                    tc.tile_pool(name="pwidx", bufs=2) as pwidxp,
                    tc.tile_pool(name="pmsg", bufs=2) as pmsgp,
                    tc.tile_pool(name="poh", bufs=2) as pohp,
                    tc.tile_pool(name="ptps", bufs=3, space="PSUM") as ptps,
                    tc.tile_pool(name="pzps", bufs=2, space="PSUM") as pzps,
                    tc.tile_pool(name="pops", bufs=2, space="PSUM") as pops,
                    tc.tile_pool(name="psb", bufs=4) as psbp,
                ):
                    plocmm_sb = pcpool.tile([P, psched["n_mm"]], dt.float16)
                    nc.scalar.dma_start(plocmm_sb[:], plocmm_in[:, :])

                    ngroups_p = psched["ngroups"]
                    Kp = psched["K"]
                    win_c0_p = psched["win_c0"]
                    msg_w = {}
                    oh_w = {}
                    for g in range(ngroups_p):
                        nch = int(Kp[g, 0])
                        c0 = int(win_c0_p[g, 0])
                        nidx = nch * P
                        idxt = pwidxp.tile([P, nidx // 16], dt.int16,
                                           tag="pidx", name=f"pidx_g{g}")
                        nc.scalar.dma_start(
                            idxt[:],
                            pgidx_in[:, c0 * P // 16:(c0 + nch) * P // 16])
                        msg = pmsgp.tile([P, nch, P], dt.float16, tag="pmsg",
                                         name=f"pmsg_g{g}")
                        nc.gpsimd.dma_gather(
                            msg[:], e_tab[:, :], idxt[:], nidx, nidx, P,
                            single_packet=False, queue_num=0)
                        oh = pohp.tile([P, nch, P], dt.float16, tag="poh",
                                       name=f"poh_g{g}")
                        nc.vector.tensor_tensor(
                            oh[:],
                            plocmm_sb[:, c0:c0 + nch].unsqueeze(2).to_broadcast(
                                [P, nch, P]),
                            iota[:, :].unsqueeze(1).to_broadcast([P, nch, P]),
                            mybir.AluOpType.is_equal,
                        )
                        msg_w[g] = (msg, c0)
                        oh_w[g] = oh

                    for k in range(PCH):
                        xt_ps = ptps.tile([P, P], dt.float32)
                        for half in range(2):
                            t = 2 * k + half
                            g = t // GROUP_P
                            msg, c0 = msg_w[g]
                            nc.tensor.matmul(
                                xt_ps[half * H:(half + 1) * H, :],
                                lhsT=msg[:, t - c0, 0:H],
                                rhs=oh_w[g][:, t - c0, :],
                                start=True, stop=True,
                            )
                        xijt = psbp.tile([P, P], dt.float16, tag="xijt")
                        nc.vector.tensor_copy(xijt[:], xt_ps[:])
                        z_ps = pzps.tile([16, P], dt.float32)
                        nc.tensor.matmul(z_ps[:], lhsT=w1_sb[:], rhs=xijt[:],
                                         start=True, stop=True)
                        z2 = psbp.tile([16, P], dt.float32, tag="z2")
                        nc.scalar.activation(
                            z2[:], z_ps[:], mybir.ActivationFunctionType.Lrelu,
                            bias=b1_sb[:, 0:1], scale=1.0, alpha=NEG)
                        o_ps = pops.tile([1, P], dt.float32)
                        nc.tensor.matmul(o_ps[:], lhsT=w2_sb[:], rhs=z2[:],
                                         start=True, stop=True)
                        osb = psbp.tile([1, P], dt.float32, tag="osb")
                        nc.scalar.activation(
                            osb[:], o_ps[:], mybir.ActivationFunctionType.Sigmoid,
                            bias=b2_sb[:, 0:1], scale=1.0)
                        nc.sync.dma_start(
                            outp[k * P:(k + 1) * P, :].rearrange("r one -> one r"),
                            osb[0:1, :])

            for _ in range(passes):
                _one_pass()

    # align each gather's SWDGE queue with its Tile-assigned DMA lane so
    # semaphore<->queue locking stays consistent (4-way parallel desc gen)
    for blk in nc.m.functions[0].blocks:
        for inst in blk.instructions:
            if isinstance(inst, mybir.InstDMAGatherAnt):
                si = inst.sync_info
                for u in (si.on_update if si else []):
                    mm = re.match(r"DMASW(\d+)_", u.ant_name or "")
                    if mm:
                        inst.queue_num = int(mm.group(1)) % 4
                        break

    nc.compile()
    return nc


def kernel(**inputs) -> np.ndarray:
    in_maps, sched = _prep(inputs)
    nc = _build(sched)
    res = run_bass_kernel_spmd(nc, in_maps, list(range(NC)))
    out = np.concatenate([res.results[c]["out"] for c in range(NC)], axis=0)
    return out.astype(np.float32)


# revision 35
# speedup vs baseline: 1.0943x; 1.0238x over previous
"""GCN message-passing kernel for 8 Trainium2 NeuronCores (Bass/Tile).

Computes (matching the jax reference):
    h = x @ W_conv                      [N, H]
    node_embed = leaky_relu(D^-1/2 (A+I) D^-1/2 h + b_conv)
    out = sigmoid(leaky(cat(e[i], e[j]) @ W1 + b1) @ W2 + b2)

Only nodes referenced by `index` (the pair head) contribute to the output,
and each core aggregates exactly the nodes its own pair block references
(~3.8K nodes, ~65K edges per core) — no second AllGather is needed; the
pair head reads the core-local embedding table. Sources come from an
AllGather (split in two, overlapped with phase A) of g = (dinv*x) @ W_conv
with dinv folded into x on the host. Edges are gathered in bulk with
dma_gather (pair-packed fp16 rows) and scatter-added on the TensorEngine
via one-hot matmuls; chunks may straddle destination tiles (one matmul per
(chunk, tile) pair) to minimize index padding, since Q7 descriptor
generation is the pacing resource.
"""

import re

import numpy as np

import concourse.bass as bass
import concourse.bacc as bacc
import concourse.mybir as mybir
import concourse.tile as tile
from concourse import library_config
from concourse.bass_utils import run_bass_kernel_spmd

NC = 8
N_NODES = 100000
F_IN = 256
H = 64
NEG = 0.01

P = 128                    # partitions / tile height
TILES = 98                 # full-node tiles per core (phase A)
SHARD = TILES * P          # 12544 nodes per core
HALF = SHARD // 2          # 6272: phase A / AllGather split granularity
NPAD = NC * SHARD          # 100352
HR = NPAD // 4             # 25088: int16-addressable pair rows per range

GROUP_E = 4                # dst tiles per edge gather group
GROUP_P = 16               # pair slot-tiles per gather window
NOMATCH = -999.0


def _node_remap(n):
    """Original node id -> row in the half-split AllGather layout."""
    c, r = n // SHARD, n % SHARD
    h = r // HALF
    return h * (NC * HALF) + c * HALF + (r - h * HALF)


def _wrap_idx(idx):
    """int array [W] (W % 16 == 0) -> [128, W//16] int16 wrapped/replicated."""
    w = idx.reshape(-1, 16).T.astype(np.int16)
    return np.tile(w, (8, 1))


def _sched2(core, tl, loc, bucket, pidx, ntiles, group_sz, nbucket):
    """Multi-tile-chunk gather/scatter schedule, uniform across cores.

    Each item is gathered from pair-packed row `pidx` (bucket selects the
    source window / rhs parity) and scatter-added to column `loc` of tile
    `tl` on core `core`. Rows are laid out per (core, group, bucket) sorted
    by (tile, pidx); chunks are consecutive 128-row windows; a chunk gets
    one matmul per tile present in ANY core's chunk (union), with per-core
    loc planes masking non-members.

    Returns (sched, idx_i16 [NC,128,totidx//16], locmm_f16 [NC,128,n_mm]).
    """
    ngroups = (ntiles + group_sz - 1) // group_sz
    grp = tl // group_sz

    wid = (core * ngroups + grp) * nbucket + bucket
    n_win = NC * ngroups * nbucket
    cnt = np.bincount(wid, minlength=n_win).reshape(NC, ngroups, nbucket)
    K = (cnt + P - 1) // P
    K = K.max(axis=0)                       # [ngroups, nbucket]

    order = np.lexsort((pidx, tl, bucket, grp, core))
    so_core, so_grp, so_b = core[order], grp[order], bucket[order]
    so_tl, so_loc, so_pidx = tl[order], loc[order], pidx[order]
    so_wid = wid[order]
    starts = np.r_[0, np.flatnonzero(np.diff(so_wid)) + 1]
    run_ids = np.zeros(len(so_wid), np.int64)
    run_ids[starts[1:]] = 1
    run_ids = np.cumsum(run_ids)
    rank = np.arange(len(so_wid)) - starts[run_ids]

    win_c0 = np.zeros((ngroups, nbucket), np.int64)
    acc = 0
    for g in range(ngroups):
        for b in range(nbucket):
            win_c0[g, b] = acc
            acc += K[g, b]
    totchunks = acc
    totidx = totchunks * P

    slot = win_c0[so_grp, so_b] * P + rank
    chunk_of = slot // P

    loc_arr = np.full((NC, totidx), -1, np.int64)
    tl_arr = np.full((NC, totidx), -1, np.int64)
    pidx_arr = np.zeros((NC, totidx), np.int64)
    loc_arr[so_core, slot] = so_loc
    tl_arr[so_core, slot] = so_tl
    pidx_arr[so_core, slot] = so_pidx

    pres = set(zip(chunk_of.tolist(), so_tl.tolist()))
    win_of_chunk = np.zeros(totchunks, np.int64)
    for g in range(ngroups):
        for b in range(nbucket):
            win_of_chunk[win_c0[g, b]: win_c0[g, b] + K[g, b]] = g * nbucket + b

    tiles_with_mm = {t for (_, t) in pres}
    for t in range(ntiles):
        if t not in tiles_with_mm:
            g = t // group_sz
            if K[g, 0] == 0:
                raise RuntimeError("empty window for dummy mm")
            pres.add((int(win_c0[g, 0]), t))

    mms = sorted(pres)
    n_mm = len(mms)

    first_mm = {}
    last_mm = {}
    for j, (c, t) in enumerate(mms):
        first_mm.setdefault(t, j)
        last_mm[t] = j

    win_mms = [[] for _ in range(ngroups * nbucket)]
    for j, (c, t) in enumerate(mms):
        win_mms[win_of_chunk[c]].append((j, c, t))

    locmm = np.full((NC, P, n_mm), NOMATCH, np.float16)
    for cc in range(NC):
        la = loc_arr[cc].reshape(totchunks, P)
        ta = tl_arr[cc].reshape(totchunks, P)
        for j, (c, t) in enumerate(mms):
            m = ta[c] == t
            if m.any():
                locmm[cc, m, j] = la[c][m].astype(np.float16)

    idx_i16 = np.zeros((NC, P, totidx // 16), np.int16)
    for cc in range(NC):
        idx_i16[cc] = _wrap_idx(pidx_arr[cc])

    sched = {
        "ngroups": ngroups,
        "nbucket": nbucket,
        "K": K,
        "win_c0": win_c0,
        "win_mms": win_mms,
        "first_mm": first_mm,
        "last_mm": last_mm,
        "n_mm": n_mm,
        "totchunks": totchunks,
        "totidx": totidx,
        "ntiles": ntiles,
        "group_sz": group_sz,
    }
    return sched, idx_i16, locmm


def _prep(inputs):
    x = np.asarray(inputs["x"], np.float32)
    edge_index = np.asarray(inputs["edge_index"], np.int64)
    index = np.asarray(inputs["index"], np.int64)
    W_conv = np.asarray(inputs["W_conv"], np.float32)
    b_conv = np.asarray(inputs["b_conv"], np.float32)
    W1 = np.asarray(inputs["W1"], np.float32)
    b1 = np.asarray(inputs["b1"], np.float32)
    W2 = np.asarray(inputs["W2"], np.float32)
    b2 = np.asarray(inputs["b2"], np.float32)

    n = x.shape[0]
    src = edge_index[0].astype(np.int64)
    dst = edge_index[1].astype(np.int64)

    deg = np.bincount(dst, minlength=NPAD).astype(np.float32) + 1.0
    deg[n:] = 1.0
    dinv = 1.0 / np.sqrt(deg)

    B = index.shape[0]
    PB = B // NC
    assert PB % P == 0
    PCH = PB // P

    # cluster pairs sharing nodes onto the same core (connected components +
    # greedy packing): ~13% fewer replicated aggregation slots. The final
    # output is un-permuted on the host.
    import scipy.sparse as _sp
    from scipy.sparse.csgraph import connected_components as _cc
    pp = np.arange(B)
    rows = np.concatenate([pp, pp])
    cols = np.concatenate([B + index[:, 0], B + index[:, 1]])
    adj = _sp.coo_matrix(
        (np.ones(2 * B, np.int8), (rows, cols)), shape=(B + n, B + n))
    _, labels = _cc(adj, directed=False)
    roots = labels[:B]
    order0 = np.argsort(roots, kind="stable")
    runs = np.r_[0, np.flatnonzero(np.diff(roots[order0])) + 1, B]
    comps = [order0[runs[i]:runs[i + 1]] for i in range(len(runs) - 1)]
    comps.sort(key=len, reverse=True)
    bins = [[] for _ in range(NC)]
    loads = [0] * NC
    for plist in comps:
        cands = [c for c in range(NC) if loads[c] + len(plist) <= PB]
        if cands:
            c = min(cands, key=lambda c: loads[c])
            bins[c].extend(plist.tolist())
            loads[c] += len(plist)
        else:
            for p in plist.tolist():
                c = min(range(NC), key=lambda c: loads[c])
                bins[c].append(p)
                loads[c] += 1
    assert loads == [PB] * NC
    pair_perm = np.arange(B, dtype=np.int64)   # clustering disabled

    # per-core needed node sets (each core aggregates what its pairs read)
    uniq_c = [np.unique(index[c * PB:(c + 1) * PB]) for c in range(NC)]
    NT_E = max(-(-len(u) // P) for u in uniq_c)
    ESHARD = NT_E * P
    slot_of = np.full((NC, n), -1, np.int64)
    for c in range(NC):
        slot_of[c, uniq_c[c]] = np.arange(len(uniq_c[c]))

    # edge stream: an edge goes to every core that needs its dst
    src_new = _node_remap(src)
    uniq_new = [_node_remap(u) for u in uniq_c]
    e_core, e_node, e_slot = [], [], []
    for c in range(NC):
        keep = slot_of[c, dst] >= 0
        e_node.append(src_new[keep])
        e_slot.append(slot_of[c, dst[keep]])
        e_core.append(np.full(keep.sum(), c, np.int64))
        # self-loops
        e_node.append(uniq_new[c])
        e_slot.append(np.arange(len(uniq_c[c]), dtype=np.int64))
        e_core.append(np.full(len(uniq_c[c]), c, np.int64))
    es = np.concatenate(e_node)
    ed = np.concatenate(e_slot)
    ec = np.concatenate(e_core)

    ebucket = 2 * (es >= 2 * HR).astype(np.int64) + (es & 1)
    epidx = (es >> 1) - (ebucket >= 2) * HR
    esched, eidx, elocmm = _sched2(
        core=ec, tl=ed // P, loc=ed % P,
        bucket=ebucket, pidx=epidx, ntiles=NT_E, group_sz=GROUP_E, nbucket=4)

    # pair stream over each core's local e-table (256B-stride rows, so the
    # slot IS the gather index: no parity buckets, zero padding); slots
    # interleaved so pair chunk k reads slot-tiles (2k, 2k+1) -> MLP
    # pipelines with the gather stream
    pair_global = np.arange(B, dtype=np.int64)
    pcore = pair_global // PB
    plocal = pair_global % PB
    s_core = np.concatenate([pcore, pcore])
    ch = plocal // P
    col = plocal % P
    s_slot = np.concatenate([(2 * ch) * P + col, (2 * ch + 1) * P + col])
    s_node = np.concatenate([
        slot_of[pcore, index[:, 0]], slot_of[pcore, index[:, 1]]])
    assert (s_node >= 0).all()
    psched, pidx, plocmm = _sched2(
        core=s_core, tl=s_slot // P, loc=s_slot % P,
        bucket=np.zeros_like(s_node), pidx=s_node,
        ntiles=2 * PCH, group_sz=GROUP_P, nbucket=1)

    # host-folded dinv: g = (dinv * x) @ W
    xpad = np.zeros((NPAD, F_IN), np.float32)
    xpad[:n] = x * dinv[:n, None]
    xT = xpad.T.astype(np.float16)
    xT_shards = [
        np.ascontiguousarray(
            xT[:, c * SHARD:(c + 1) * SHARD].reshape(2, P, SHARD).transpose(1, 0, 2)
        ) for c in range(NC)
    ]
    # dinv over each core's local e-slot space, and bconv/dinv bias tables
    # (added into the PSUM accumulator via an identity matmul so the whole
    # consume is one Lrelu activation)
    dinv_e_sb = []
    bde_sb = []
    for c in range(NC):
        d = np.zeros(ESHARD, np.float32)
        d[:len(uniq_c[c])] = dinv[uniq_c[c]]
        dinv_e_sb.append(np.ascontiguousarray(d.reshape(NT_E, P).T))
        bde = np.zeros((ESHARD, H), np.float32)
        nu = len(uniq_c[c])
        bde[:nu] = b_conv[None, :] / d[:nu, None]
        bde_sb.append(np.ascontiguousarray(
            bde.reshape(NT_E, P, H).transpose(1, 0, 2)).astype(np.float16))

    consts = {
        "wc": np.ascontiguousarray(
            W_conv.reshape(2, P, H).transpose(1, 0, 2)).astype(np.float16),
        "iota": np.broadcast_to(np.arange(P, dtype=np.float16), (P, P)).copy(),
        "ident": np.eye(P, dtype=np.float16),
        "w1": W1.astype(np.float16),
        "b1": b1.reshape(16, 1).astype(np.float32),
        "w2": W2.astype(np.float32),
        "b2t": b2.reshape(1, 1).astype(np.float32),
    }
    sched = {"edge": esched, "pair": psched, "PCH": PCH,
             "NT_E": NT_E, "ESHARD": ESHARD, "pair_perm": pair_perm}
    in_maps = []
    for c in range(NC):
        m = {
            "xt": xT_shards[c],
            "dinve": dinv_e_sb[c],
            "bde": bde_sb[c],
            "elocmm": elocmm[c],
            "egidx": eidx[c],
            "plocmm": plocmm[c],
            "pgidx": pidx[c],
        }
        m.update(consts)
        in_maps.append(m)
    return in_maps, sched


def _emit_scatter2(nc, dt, src_aps, idx_dram, locmm_sb, iota, sched,
                   pools, consume, prefix, post_group=None, idx_eng=None,
                   acc_init=None):
    """Gather pair-packed rows per window, build per-matmul one-hot planes,
    matmul-accumulate into per-tile PSUM, hand finished tiles to consume.

    src_aps: bucket -> source AP (pair-packed rows).
    """
    widxp, msgp, ohp, accp = pools
    if idx_eng is None:
        idx_eng = nc.sync
    ngroups = sched["ngroups"]
    nbucket = sched["nbucket"]
    K = sched["K"]
    win_c0 = sched["win_c0"]
    win_mms = sched["win_mms"]
    first_mm = sched["first_mm"]
    last_mm = sched["last_mm"]
    group_sz = sched["group_sz"]
    ntiles = sched["ntiles"]

    acc_tiles = {}

    def _drain(g):
        # consume runs one group late so the DVE queue doesn't head-block
        # on PSUM drains while later windows' one-hots are still pending
        for t in range(g * group_sz, min((g + 1) * group_sz, ntiles)):
            consume(t, acc_tiles.pop(t))
        if post_group is not None:
            post_group(g)

    for g in range(ngroups):
        for b in range(nbucket):
            nch = int(K[g, b])
            if nch == 0:
                continue
            c0 = int(win_c0[g, b])
            nidx = nch * P
            idxt = widxp.tile([P, nidx // 16], dt.int16, tag=f"{prefix}idx{b}",
                              name=f"{prefix}idx_g{g}b{b}")
            idx_eng.dma_start(
                idxt[:], idx_dram[:, c0 * P // 16:(c0 + nch) * P // 16])
            msg = msgp.tile([P, nch, P], dt.float16, tag=f"{prefix}msg{b}",
                            name=f"{prefix}msg_g{g}b{b}")
            nc.gpsimd.dma_gather(
                msg[:], src_aps[b], idxt[:], nidx, nidx, P,
                single_packet=False, queue_num=0)
            mms = win_mms[g * nbucket + b]
            if not mms:
                continue
            j0 = mms[0][0]
            n_mm_w = len(mms)
            oh = ohp.tile([P, n_mm_w, P], dt.float16, tag=f"{prefix}oh{b}",
                          name=f"{prefix}oh_g{g}b{b}")
            nc.vector.tensor_tensor(
                oh[:],
                locmm_sb[:, j0:j0 + n_mm_w].unsqueeze(2).to_broadcast(
                    [P, n_mm_w, P]),
                iota[:, :].unsqueeze(1).to_broadcast([P, n_mm_w, P]),
                mybir.AluOpType.is_equal,
            )
            par = b & 1
            for (j, c, t) in mms:
                if t not in acc_tiles:
                    acc_tiles[t] = accp.tile(
                        [P, H], dt.float32, tag=f"{prefix}acc{t % group_sz}",
                        name=f"{prefix}acc_t{t}")
                    if acc_init is not None:
                        acc_init(t, acc_tiles[t])
                nc.tensor.matmul(
                    acc_tiles[t][:],
                    lhsT=oh[:, j - j0, :],
                    rhs=msg[:, c - c0, par * H:(par + 1) * H],
                    start=(acc_init is None and j == first_mm[t]),
                    stop=(j == last_mm[t]),
                )
        if g > 0:
            _drain(g - 1)
    _drain(ngroups - 1)


def _build(sched, passes=1):
    dt = mybir.dt
    esched = sched["edge"]
    psched = sched["pair"]
    PCH = sched["PCH"]
    NT_E = sched["NT_E"]
    ESHARD = sched["ESHARD"]

    nc = bacc.Bacc("TRN2", target_bir_lowering=False, debug=False,
                   enable_asserts=False, num_devices=NC, num_swdge_queues=4)

    xt_in = nc.dram_tensor("xt", [P, 2, SHARD], dt.float16, kind="ExternalInput")
    dinve_in = nc.dram_tensor("dinve", [P, NT_E], dt.float32,
                              kind="ExternalInput")
    bde_in = nc.dram_tensor("bde", [P, NT_E, H], dt.float16,
                            kind="ExternalInput")
    elocmm_in = nc.dram_tensor("elocmm", [P, esched["n_mm"]], dt.float16,
                               kind="ExternalInput")
    egidx_in = nc.dram_tensor("egidx", [P, esched["totidx"] // 16], dt.int16,
                              kind="ExternalInput")
    plocmm_in = nc.dram_tensor("plocmm", [P, psched["n_mm"]], dt.float16,
                               kind="ExternalInput")
    pgidx_in = nc.dram_tensor("pgidx", [P, psched["totidx"] // 16], dt.int16,
                              kind="ExternalInput")
    wc_in = nc.dram_tensor("wc", [P, 2, H], dt.float16, kind="ExternalInput")
    iota_in = nc.dram_tensor("iota", [P, P], dt.float16, kind="ExternalInput")
    ident_in = nc.dram_tensor("ident", [P, P], dt.float16, kind="ExternalInput")
    w1_in = nc.dram_tensor("w1", [P, 16], dt.float16, kind="ExternalInput")
    b1_in = nc.dram_tensor("b1", [16, 1], dt.float32, kind="ExternalInput")
    w2_in = nc.dram_tensor("w2", [16, 1], dt.float32, kind="ExternalInput")
    b2_in = nc.dram_tensor("b2t", [1, 1], dt.float32, kind="ExternalInput")
    outp = nc.dram_tensor("out", [PCH * P, 1], dt.float32, kind="ExternalOutput")

    g_shard = nc.dram_tensor("g_shard", [SHARD, H], dt.float16)
    g_full = nc.dram_tensor("g_full", [NPAD, H], dt.float16, addr_space="Shared")
    # local e-table with 256B row stride; only the first H columns are
    # written/read (the tail pads rows to dma_gather's 256B granularity)
    e_tab = nc.dram_tensor("e_tab", [ESHARD, 2 * H], dt.float16)

    g_pairs = g_full[:, :].rearrange("(r two) f -> r (two f)", two=2)

    with tile.TileContext(nc) as tc:
        nc.gpsimd.load_library(library_config.mlp)

        with (
            tc.tile_pool(name="const", bufs=1) as cpool,
            tc.tile_pool(name="dinvp", bufs=1) as dpool,
        ):
            wc_sb = cpool.tile([P, 2, H], dt.float16)
            nc.sync.dma_start(wc_sb[:], wc_in[:, :, :])
            iota = cpool.tile([P, P], dt.float16)
            nc.sync.dma_start(iota[:], iota_in[:, :])
            ident = cpool.tile([P, P], dt.float16)
            nc.sync.dma_start(ident[:], ident_in[:, :])
            w1_sb = cpool.tile([P, 16], dt.float16)
            nc.sync.dma_start(w1_sb[:], w1_in[:, :])
            b1_sb = cpool.tile([16, 1], dt.float32)
            nc.sync.dma_start(b1_sb[:], b1_in[:, :])
            w2_sb = cpool.tile([16, 1], dt.float32)
            nc.sync.dma_start(w2_sb[:], w2_in[:, :])
            b2_sb = cpool.tile([1, 1], dt.float32)
            nc.sync.dma_start(b2_sb[:], b2_in[:, :])
            elocmm_sb = cpool.tile([P, esched["n_mm"]], dt.float16)
            nc.sync.dma_start(elocmm_sb[:], elocmm_in[:, :])

            dinve = dpool.tile([P, NT_E], dt.float32)
            nc.sync.dma_start(dinve[:], dinve_in[:, :])
            bde_sb = dpool.tile([P, NT_E, H], dt.float16)
            nc.sync.dma_start(bde_sb[:], bde_in[:, :, :])


            def _one_pass():
                # ------- phase A: g = x' @ W, AllGather split in halves -------
                XBLK = 7
                HTILES = HALF // P        # tiles 0..48 cover rows [0, HALF)
                assert HTILES % XBLK == 0
                with (
                    tc.tile_pool(name="xtp", bufs=2) as xtp,
                    tc.tile_pool(name="hps", bufs=4, space="PSUM") as hps,
                    tc.tile_pool(name="gsb", bufs=1) as gsbp,
                ):
                    g_sb = gsbp.tile([P, TILES, H], dt.float16)
                    for blk in range((TILES + XBLK - 1) // XBLK):
                        t0, t1 = blk * XBLK, min((blk + 1) * XBLK, TILES)
                        xt_sb = xtp.tile([P, 2, (t1 - t0) * P], dt.float16, tag="xt")
                        nc.sync.dma_start(xt_sb[:], xt_in[:, :, t0 * P: t1 * P])
                        for t in range(t0, t1):
                            h_ps = hps.tile([P, H], dt.float32)
                            for k in range(2):
                                nc.tensor.matmul(
                                    h_ps[:],
                                    lhsT=xt_sb[:, k, (t - t0) * P:(t - t0 + 1) * P],
                                    rhs=wc_sb[:, k, :],
                                    start=(k == 0), stop=(k == 1),
                                )
                            nc.scalar.activation(
                                g_sb[:, t, :], h_ps[:],
                                mybir.ActivationFunctionType.Copy,
                                bias=0.0, scale=1.0)
                        if t1 == HTILES:
                            # first half ready: store + gather while 2nd half runs
                            nc.sync.dma_start(
                                g_shard[0:HALF, :].rearrange(
                                    "(t p) f -> p t f", p=P),
                                g_sb[:, 0:HALF // P, :],
                            )
                            nc.gpsimd.collective_compute(
                                "AllGather", mybir.AluOpType.bypass,
                                replica_groups=[list(range(NC))],
                                ins=[g_shard[0:HALF, :].opt()],
                                outs=[g_full[0:NC * HALF, :].opt()],
                            )
                    nc.sync.dma_start(
                        g_shard[HALF:, :].rearrange("(t p) f -> p t f", p=P),
                        g_sb[:, HALF // P:, :],
                    )

                nc.gpsimd.collective_compute(
                    "AllGather", mybir.AluOpType.bypass,
                    replica_groups=[list(range(NC))],
                    ins=[g_shard[HALF:, :].opt()],
                    outs=[g_full[NC * HALF:, :].opt()],
                )

                # ---------------- phase C: aggregate per dst tile ----------------
                with (
                    tc.tile_pool(name="ewidx", bufs=3) as widxp,
                    tc.tile_pool(name="emsg", bufs=3) as msgp,
                    tc.tile_pool(name="eoh", bufs=3) as ohp,
                    tc.tile_pool(name="eacc", bufs=2, space="PSUM") as accp,
                    tc.tile_pool(name="eemb", bufs=4) as embp,
                ):
                    def acc_init_edge(t, a):
                        # seed PSUM with bconv/dinv so consume is one Lrelu
                        nc.tensor.matmul(a[:], lhsT=ident[:],
                                         rhs=bde_sb[:, t, :],
                                         start=True, stop=False)

                    def consume_edge(t, a):
                        emb = embp.tile([P, H], dt.float16, name=f"emb_{t}")
                        nc.scalar.activation(
                            emb[:], a[:], mybir.ActivationFunctionType.Lrelu,
                            bias=0.0, scale=dinve[:, t:t + 1], alpha=NEG)
                        nc.sync.dma_start(e_tab[t * P:(t + 1) * P, 0:H], emb[:])

                    e_src_aps = [g_pairs, g_pairs,
                                 g_pairs[HR:, :], g_pairs[HR:, :]]
                    _emit_scatter2(nc, dt, e_src_aps, egidx_in, elocmm_sb, iota,
                                   esched, (widxp, msgp, ohp, accp),
                                   consume_edge, "e", acc_init=acc_init_edge)

                # ------- phase D: pair gather + permute-matmul + MLP -------
                # pair slots are laid out so gather chunk == slot tile and
                # tiles (2k, 2k+1) hold pair-chunk k's xi/xj lookups; one
                # matmul msg.T @ onehot produces each 64-row half of xijt
                # (feature-major, pair-ordered) straight from the gather.
                with (
                    tc.tile_pool(name="pconst", bufs=1) as pcpool,
                    tc.tile_pool(name="pwidx", bufs=2) as pwidxp,
                    tc.tile_pool(name="pmsg", bufs=2) as pmsgp,
                    tc.tile_pool(name="poh", bufs=2) as pohp,
                    tc.tile_pool(name="ptps", bufs=3, space="PSUM") as ptps,
                    tc.tile_pool(name="pzps", bufs=2, space="PSUM") as pzps,
                    tc.tile_pool(name="pops", bufs=2, space="PSUM") as pops,
                    tc.tile_pool(name="psb", bufs=4) as psbp,
                ):
                    plocmm_sb = pcpool.tile([P, psched["n_mm"]], dt.float16)
                    nc.scalar.dma_start(plocmm_sb[:], plocmm_in[:, :])

                    ngroups_p = psched["ngroups"]
                    Kp = psched["K"]
                    win_c0_p = psched["win_c0"]
                    msg_w = {}
                    oh_w = {}
                    for g in range(ngroups_p):
                        nch = int(Kp[g, 0])
                        c0 = int(win_c0_p[g, 0])
                        nidx = nch * P
                        idxt = pwidxp.tile([P, nidx // 16], dt.int16,
                                           tag="pidx", name=f"pidx_g{g}")
                        nc.scalar.dma_start(
                            idxt[:],
                            pgidx_in[:, c0 * P // 16:(c0 + nch) * P // 16])
                        msg = pmsgp.tile([P, nch, P], dt.float16, tag="pmsg",
                                         name=f"pmsg_g{g}")
                        nc.gpsimd.dma_gather(
                            msg[:], e_tab[:, :], idxt[:], nidx, nidx, P,
                            single_packet=False, queue_num=0)
                        oh = pohp.tile([P, nch, P], dt.float16, tag="poh",
                                       name=f"poh_g{g}")
                        nc.vector.tensor_tensor(
                            oh[:],
                            plocmm_sb[:, c0:c0 + nch].unsqueeze(2).to_broadcast(
                                [P, nch, P]),
                            iota[:, :].unsqueeze(1).to_broadcast([P, nch, P]),
                            mybir.AluOpType.is_equal,
                        )
                        msg_w[g] = (msg, c0)
                        oh_w[g] = oh

                    for k in range(PCH):
                        xt_ps = ptps.tile([P, P], dt.float32)
                        for half in range(2):
                            t = 2 * k + half
                            g = t // GROUP_P
                            msg, c0 = msg_w[g]
                            nc.tensor.matmul(
                                xt_ps[half * H:(half + 1) * H, :],
                                lhsT=msg[:, t - c0, 0:H],
                                rhs=oh_w[g][:, t - c0, :],
                                start=True, stop=True,
                            )
                        xijt = psbp.tile([P, P], dt.float16, tag="xijt")
                        nc.vector.tensor_copy(xijt[:], xt_ps[:])
                        z_ps = pzps.tile([16, P], dt.float32)
                        nc.tensor.matmul(z_ps[:], lhsT=w1_sb[:], rhs=xijt[:],
                                         start=True, stop=True)
                        z2 = psbp.tile([16, P], dt.float32, tag="z2")
                        nc.scalar.activation(
                            z2[:], z_ps[:], mybir.ActivationFunctionType.Lrelu,
                            bias=b1_sb[:, 0:1], scale=1.0, alpha=NEG)
                        o_ps = pops.tile([1, P], dt.float32)
                        nc.tensor.matmul(o_ps[:], lhsT=w2_sb[:], rhs=z2[:],
                                         start=True, stop=True)
                        osb = psbp.tile([1, P], dt.float32, tag="osb")
                        nc.scalar.activation(
                            osb[:], o_ps[:], mybir.ActivationFunctionType.Sigmoid,
                            bias=b2_sb[:, 0:1], scale=1.0)
                        nc.sync.dma_start(
                            outp[k * P:(k + 1) * P, :].rearrange("r one -> one r"),
                            osb[0:1, :])

            for _ in range(passes):
                _one_pass()

    # align each gather's SWDGE queue with its Tile-assigned DMA lane so
    # semaphore<->queue locking stays consistent (4-way parallel desc gen)
    for blk in nc.m.functions[0].blocks:
        for inst in blk.instructions:
            if isinstance(inst, mybir.InstDMAGatherAnt):
                si = inst.sync_info
                for u in (si.on_update if si else []):
                    mm = re.match(r"DMASW(\d+)_", u.ant_name or "")
                    if mm:
                        inst.queue_num = int(mm.group(1)) % 4
                        break

    nc.compile()
    return nc


def kernel(**inputs) -> np.ndarray:
    in_maps, sched = _prep(inputs)
    nc = _build(sched)
    res = run_bass_kernel_spmd(nc, in_maps, list(range(NC)))
    out = np.concatenate([res.results[c]["out"] for c in range(NC)], axis=0)
    full = np.empty_like(out)
    full[sched["pair_perm"]] = out
    return full.astype(np.float32)


# revision 36
# speedup vs baseline: 1.1032x; 1.0081x over previous
"""GCN message-passing kernel for 8 Trainium2 NeuronCores (Bass/Tile).

Computes (matching the jax reference):
    h = x @ W_conv                      [N, H]
    node_embed = leaky_relu(D^-1/2 (A+I) D^-1/2 h + b_conv)
    out = sigmoid(leaky(cat(e[i], e[j]) @ W1 + b1) @ W2 + b2)

Only nodes referenced by `index` (the pair head) contribute to the output,
and each core aggregates exactly the nodes its own pair block references
(~3.8K nodes, ~65K edges per core) — no second AllGather is needed; the
pair head reads the core-local embedding table. Sources come from an
AllGather (split in two, overlapped with phase A) of g = (dinv*x) @ W_conv
with dinv folded into x on the host. Edges are gathered in bulk with
dma_gather (pair-packed fp16 rows) and scatter-added on the TensorEngine
via one-hot matmuls; chunks may straddle destination tiles (one matmul per
(chunk, tile) pair) to minimize index padding, since Q7 descriptor
generation is the pacing resource.
"""

import re

import numpy as np

import concourse.bass as bass
import concourse.bacc as bacc
import concourse.mybir as mybir
import concourse.tile as tile
from concourse import library_config
from concourse.bass_utils import run_bass_kernel_spmd

NC = 8
N_NODES = 100000
F_IN = 256
H = 64
NEG = 0.01

P = 128                    # partitions / tile height
TILES = 98                 # full-node tiles per core (phase A)
SHARD = TILES * P          # 12544 nodes per core
HALF = SHARD // 2          # 6272: phase A / AllGather split granularity
NPAD = NC * SHARD          # 100352
HR = NPAD // 4             # 25088: int16-addressable pair rows per range

GROUP_E = 4                # dst tiles per edge gather group
GROUP_P = 16               # pair slot-tiles per gather window
NOMATCH = -999.0


def _node_remap(n):
    """Original node id -> row in the half-split AllGather layout."""
    c, r = n // SHARD, n % SHARD
    h = r // HALF
    return h * (NC * HALF) + c * HALF + (r - h * HALF)


def _wrap_idx(idx):
    """int array [W] (W % 16 == 0) -> [128, W//16] int16 wrapped/replicated."""
    w = idx.reshape(-1, 16).T.astype(np.int16)
    return np.tile(w, (8, 1))


def _sched2(core, tl, loc, bucket, pidx, ntiles, group_sz, nbucket):
    """Multi-tile-chunk gather/scatter schedule, uniform across cores.

    Each item is gathered from pair-packed row `pidx` (bucket selects the
    source window / rhs parity) and scatter-added to column `loc` of tile
    `tl` on core `core`. Rows are laid out per (core, group, bucket) sorted
    by (tile, pidx); chunks are consecutive 128-row windows; a chunk gets
    one matmul per tile present in ANY core's chunk (union), with per-core
    loc planes masking non-members.

    Returns (sched, idx_i16 [NC,128,totidx//16], locmm_f16 [NC,128,n_mm]).
    """
    ngroups = (ntiles + group_sz - 1) // group_sz
    grp = tl // group_sz

    wid = (core * ngroups + grp) * nbucket + bucket
    n_win = NC * ngroups * nbucket
    cnt = np.bincount(wid, minlength=n_win).reshape(NC, ngroups, nbucket)
    K = (cnt + P - 1) // P
    K = K.max(axis=0)                       # [ngroups, nbucket]

    order = np.lexsort((pidx, tl, bucket, grp, core))
    so_core, so_grp, so_b = core[order], grp[order], bucket[order]
    so_tl, so_loc, so_pidx = tl[order], loc[order], pidx[order]
    so_wid = wid[order]
    starts = np.r_[0, np.flatnonzero(np.diff(so_wid)) + 1]
    run_ids = np.zeros(len(so_wid), np.int64)
    run_ids[starts[1:]] = 1
    run_ids = np.cumsum(run_ids)
    rank = np.arange(len(so_wid)) - starts[run_ids]

    win_c0 = np.zeros((ngroups, nbucket), np.int64)
    acc = 0
    for g in range(ngroups):
        for b in range(nbucket):
            win_c0[g, b] = acc
            acc += K[g, b]
    totchunks = acc
    totidx = totchunks * P

    slot = win_c0[so_grp, so_b] * P + rank
    chunk_of = slot // P

    loc_arr = np.full((NC, totidx), -1, np.int64)
    tl_arr = np.full((NC, totidx), -1, np.int64)
    pidx_arr = np.zeros((NC, totidx), np.int64)
    loc_arr[so_core, slot] = so_loc
    tl_arr[so_core, slot] = so_tl
    pidx_arr[so_core, slot] = so_pidx

    pres = set(zip(chunk_of.tolist(), so_tl.tolist()))
    win_of_chunk = np.zeros(totchunks, np.int64)
    for g in range(ngroups):
        for b in range(nbucket):
            win_of_chunk[win_c0[g, b]: win_c0[g, b] + K[g, b]] = g * nbucket + b

    tiles_with_mm = {t for (_, t) in pres}
    for t in range(ntiles):
        if t not in tiles_with_mm:
            g = t // group_sz
            if K[g, 0] == 0:
                raise RuntimeError("empty window for dummy mm")
            pres.add((int(win_c0[g, 0]), t))

    mms = sorted(pres)
    n_mm = len(mms)

    first_mm = {}
    last_mm = {}
    for j, (c, t) in enumerate(mms):
        first_mm.setdefault(t, j)
        last_mm[t] = j

    win_mms = [[] for _ in range(ngroups * nbucket)]
    for j, (c, t) in enumerate(mms):
        win_mms[win_of_chunk[c]].append((j, c, t))

    locmm = np.full((NC, P, n_mm), NOMATCH, np.float16)
    for cc in range(NC):
        la = loc_arr[cc].reshape(totchunks, P)
        ta = tl_arr[cc].reshape(totchunks, P)
        for j, (c, t) in enumerate(mms):
            m = ta[c] == t
            if m.any():
                locmm[cc, m, j] = la[c][m].astype(np.float16)

    idx_i16 = np.zeros((NC, P, totidx // 16), np.int16)
    for cc in range(NC):
        idx_i16[cc] = _wrap_idx(pidx_arr[cc])

    sched = {
        "ngroups": ngroups,
        "nbucket": nbucket,
        "K": K,
        "win_c0": win_c0,
        "win_mms": win_mms,
        "first_mm": first_mm,
        "last_mm": last_mm,
        "n_mm": n_mm,
        "totchunks": totchunks,
        "totidx": totidx,
        "ntiles": ntiles,
        "group_sz": group_sz,
    }
    return sched, idx_i16, locmm


def _prep(inputs):
    x = np.asarray(inputs["x"], np.float32)
    edge_index = np.asarray(inputs["edge_index"], np.int64)
    index = np.asarray(inputs["index"], np.int64)
    W_conv = np.asarray(inputs["W_conv"], np.float32)
    b_conv = np.asarray(inputs["b_conv"], np.float32)
    W1 = np.asarray(inputs["W1"], np.float32)
    b1 = np.asarray(inputs["b1"], np.float32)
    W2 = np.asarray(inputs["W2"], np.float32)
    b2 = np.asarray(inputs["b2"], np.float32)

    n = x.shape[0]
    src = edge_index[0].astype(np.int64)
    dst = edge_index[1].astype(np.int64)

    deg = np.bincount(dst, minlength=NPAD).astype(np.float32) + 1.0
    deg[n:] = 1.0
    dinv = 1.0 / np.sqrt(deg)

    B = index.shape[0]
    PB = B // NC
    assert PB % P == 0
    PCH = PB // P

    # cluster pairs sharing nodes onto the same core (connected components +
    # greedy packing): ~13% fewer replicated aggregation slots. The final
    # output is un-permuted on the host.
    import scipy.sparse as _sp
    from scipy.sparse.csgraph import connected_components as _cc
    pp = np.arange(B)
    rows = np.concatenate([pp, pp])
    cols = np.concatenate([B + index[:, 0], B + index[:, 1]])
    adj = _sp.coo_matrix(
        (np.ones(2 * B, np.int8), (rows, cols)), shape=(B + n, B + n))
    _, labels = _cc(adj, directed=False)
    roots = labels[:B]
    order0 = np.argsort(roots, kind="stable")
    runs = np.r_[0, np.flatnonzero(np.diff(roots[order0])) + 1, B]
    comps = [order0[runs[i]:runs[i + 1]] for i in range(len(runs) - 1)]
    comps.sort(key=len, reverse=True)
    bins = [[] for _ in range(NC)]
    loads = [0] * NC
    for plist in comps:
        cands = [c for c in range(NC) if loads[c] + len(plist) <= PB]
        if cands:
            c = min(cands, key=lambda c: loads[c])
            bins[c].extend(plist.tolist())
            loads[c] += len(plist)
        else:
            for p in plist.tolist():
                c = min(range(NC), key=lambda c: loads[c])
                bins[c].append(p)
                loads[c] += 1
    assert loads == [PB] * NC
    pair_perm = np.arange(B, dtype=np.int64)   # clustering disabled

    # per-core needed node sets (each core aggregates what its pairs read)
    uniq_c = [np.unique(index[c * PB:(c + 1) * PB]) for c in range(NC)]
    NT_E = max(-(-len(u) // P) for u in uniq_c)
    ESHARD = NT_E * P
    slot_of = np.full((NC, n), -1, np.int64)
    for c in range(NC):
        slot_of[c, uniq_c[c]] = np.arange(len(uniq_c[c]))

    # edge stream: an edge goes to every core that needs its dst
    src_new = _node_remap(src)
    uniq_new = [_node_remap(u) for u in uniq_c]
    e_core, e_node, e_slot = [], [], []
    for c in range(NC):
        keep = slot_of[c, dst] >= 0
        e_node.append(src_new[keep])
        e_slot.append(slot_of[c, dst[keep]])
        e_core.append(np.full(keep.sum(), c, np.int64))
        # self-loops
        e_node.append(uniq_new[c])
        e_slot.append(np.arange(len(uniq_c[c]), dtype=np.int64))
        e_core.append(np.full(len(uniq_c[c]), c, np.int64))
    es = np.concatenate(e_node)
    ed = np.concatenate(e_slot)
    ec = np.concatenate(e_core)

    ebucket = 2 * (es >= 2 * HR).astype(np.int64) + (es & 1)
    epidx = (es >> 1) - (ebucket >= 2) * HR
    esched, eidx, elocmm = _sched2(
        core=ec, tl=ed // P, loc=ed % P,
        bucket=ebucket, pidx=epidx, ntiles=NT_E, group_sz=GROUP_E, nbucket=4)

    # pair stream over each core's local e-table (256B-stride rows, so the
    # slot IS the gather index: no parity buckets, zero padding); slots
    # interleaved so pair chunk k reads slot-tiles (2k, 2k+1) -> MLP
    # pipelines with the gather stream
    pair_global = np.arange(B, dtype=np.int64)
    pcore = pair_global // PB
    plocal = pair_global % PB
    s_core = np.concatenate([pcore, pcore])
    ch = plocal // P
    col = plocal % P
    s_slot = np.concatenate([(2 * ch) * P + col, (2 * ch + 1) * P + col])
    s_node = np.concatenate([
        slot_of[pcore, index[:, 0]], slot_of[pcore, index[:, 1]]])
    assert (s_node >= 0).all()
    psched, pidx, plocmm = _sched2(
        core=s_core, tl=s_slot // P, loc=s_slot % P,
        bucket=np.zeros_like(s_node), pidx=s_node,
        ntiles=2 * PCH, group_sz=GROUP_P, nbucket=1)

    # host-folded dinv: g = (dinv * x) @ W
    xpad = np.zeros((NPAD, F_IN), np.float32)
    xpad[:n] = x * dinv[:n, None]
    xT = xpad.T.astype(np.float16)
    xT_shards = [
        np.ascontiguousarray(
            xT[:, c * SHARD:(c + 1) * SHARD].reshape(2, P, SHARD).transpose(1, 0, 2)
        ) for c in range(NC)
    ]
    # dinv over each core's local e-slot space, and bconv/dinv bias tables
    # (added into the PSUM accumulator via an identity matmul so the whole
    # consume is one Lrelu activation)
    dinv_e_sb = []
    bde_sb = []
    for c in range(NC):
        d = np.zeros(ESHARD, np.float32)
        d[:len(uniq_c[c])] = dinv[uniq_c[c]]
        dinv_e_sb.append(np.ascontiguousarray(d.reshape(NT_E, P).T))
        bde = np.zeros((ESHARD, H), np.float32)
        nu = len(uniq_c[c])
        bde[:nu] = b_conv[None, :] / d[:nu, None]
        bde_sb.append(np.ascontiguousarray(
            bde.reshape(NT_E, P, H).transpose(1, 0, 2)).astype(np.float16))

    consts = {
        "wc": np.ascontiguousarray(
            W_conv.reshape(2, P, H).transpose(1, 0, 2)).astype(np.float16),
        "iota": np.broadcast_to(np.arange(P, dtype=np.float16), (P, P)).copy(),
        "ident": np.eye(P, dtype=np.float16),
        "w1": W1.astype(np.float16),
        "b1": b1.reshape(16, 1).astype(np.float32),
        "w2": W2.astype(np.float32),
        "b2t": b2.reshape(1, 1).astype(np.float32),
    }
    sched = {"edge": esched, "pair": psched, "PCH": PCH,
             "NT_E": NT_E, "ESHARD": ESHARD, "pair_perm": pair_perm}
    in_maps = []
    for c in range(NC):
        m = {
            "xt": xT_shards[c],
            "dinve": dinv_e_sb[c],
            "bde": bde_sb[c],
            "elocmm": elocmm[c],
            "egidx": eidx[c],
            "plocmm": plocmm[c],
            "pgidx": pidx[c],
        }
        m.update(consts)
        in_maps.append(m)
    return in_maps, sched


def _emit_scatter2(nc, dt, src_aps, idx_dram, locmm_sb, iota, sched,
                   pools, consume, prefix, post_group=None, idx_eng=None,
                   acc_init=None):
    """Gather pair-packed rows per window, build per-matmul one-hot planes,
    matmul-accumulate into per-tile PSUM, hand finished tiles to consume.

    src_aps: bucket -> source AP (pair-packed rows).
    """
    widxp, msgp, ohp, accp = pools
    if idx_eng is None:
        idx_eng = nc.sync
    ngroups = sched["ngroups"]
    nbucket = sched["nbucket"]
    K = sched["K"]
    win_c0 = sched["win_c0"]
    win_mms = sched["win_mms"]
    first_mm = sched["first_mm"]
    last_mm = sched["last_mm"]
    group_sz = sched["group_sz"]
    ntiles = sched["ntiles"]

    acc_tiles = {}

    def _drain(g):
        # consume runs one group late so the DVE queue doesn't head-block
        # on PSUM drains while later windows' one-hots are still pending
        for t in range(g * group_sz, min((g + 1) * group_sz, ntiles)):
            consume(t, acc_tiles.pop(t))
        if post_group is not None:
            post_group(g)

    for g in range(ngroups):
        for b in range(nbucket):
            nch = int(K[g, b])
            if nch == 0:
                continue
            c0 = int(win_c0[g, b])
            nidx = nch * P
            idxt = widxp.tile([P, nidx // 16], dt.int16, tag=f"{prefix}idx{b}",
                              name=f"{prefix}idx_g{g}b{b}")
            idx_eng.dma_start(
                idxt[:], idx_dram[:, c0 * P // 16:(c0 + nch) * P // 16])
            msg = msgp.tile([P, nch, P], dt.float16, tag=f"{prefix}msg{b}",
                            name=f"{prefix}msg_g{g}b{b}")
            nc.gpsimd.dma_gather(
                msg[:], src_aps[b], idxt[:], nidx, nidx, P,
                single_packet=False, queue_num=0)
            mms = win_mms[g * nbucket + b]
            if not mms:
                continue
            j0 = mms[0][0]
            n_mm_w = len(mms)
            oh = ohp.tile([P, n_mm_w, P], dt.float16, tag=f"{prefix}oh{b}",
                          name=f"{prefix}oh_g{g}b{b}")
            nc.vector.tensor_tensor(
                oh[:],
                locmm_sb[:, j0:j0 + n_mm_w].unsqueeze(2).to_broadcast(
                    [P, n_mm_w, P]),
                iota[:, :].unsqueeze(1).to_broadcast([P, n_mm_w, P]),
                mybir.AluOpType.is_equal,
            )
            par = b & 1
            for (j, c, t) in mms:
                if t not in acc_tiles:
                    acc_tiles[t] = accp.tile(
                        [P, H], dt.float32, tag=f"{prefix}acc{t % group_sz}",
                        name=f"{prefix}acc_t{t}")
                    if acc_init is not None:
                        acc_init(t, acc_tiles[t])
                nc.tensor.matmul(
                    acc_tiles[t][:],
                    lhsT=oh[:, j - j0, :],
                    rhs=msg[:, c - c0, par * H:(par + 1) * H],
                    start=(acc_init is None and j == first_mm[t]),
                    stop=(j == last_mm[t]),
                )
        if g > 0:
            _drain(g - 1)
    _drain(ngroups - 1)


def _build(sched, passes=1):
    dt = mybir.dt
    esched = sched["edge"]
    psched = sched["pair"]
    PCH = sched["PCH"]
    NT_E = sched["NT_E"]
    ESHARD = sched["ESHARD"]

    nc = bacc.Bacc("TRN2", target_bir_lowering=False, debug=False,
                   enable_asserts=False, num_devices=NC, num_swdge_queues=4)

    xt_in = nc.dram_tensor("xt", [P, 2, SHARD], dt.float16, kind="ExternalInput")
    dinve_in = nc.dram_tensor("dinve", [P, NT_E], dt.float32,
                              kind="ExternalInput")
    bde_in = nc.dram_tensor("bde", [P, NT_E, H], dt.float16,
                            kind="ExternalInput")
    elocmm_in = nc.dram_tensor("elocmm", [P, esched["n_mm"]], dt.float16,
                               kind="ExternalInput")
    egidx_in = nc.dram_tensor("egidx", [P, esched["totidx"] // 16], dt.int16,
                              kind="ExternalInput")
    plocmm_in = nc.dram_tensor("plocmm", [P, psched["n_mm"]], dt.float16,
                               kind="ExternalInput")
    pgidx_in = nc.dram_tensor("pgidx", [P, psched["totidx"] // 16], dt.int16,
                              kind="ExternalInput")
    wc_in = nc.dram_tensor("wc", [P, 2, H], dt.float16, kind="ExternalInput")
    iota_in = nc.dram_tensor("iota", [P, P], dt.float16, kind="ExternalInput")
    ident_in = nc.dram_tensor("ident", [P, P], dt.float16, kind="ExternalInput")
    w1_in = nc.dram_tensor("w1", [P, 16], dt.float16, kind="ExternalInput")
    b1_in = nc.dram_tensor("b1", [16, 1], dt.float32, kind="ExternalInput")
    w2_in = nc.dram_tensor("w2", [16, 1], dt.float32, kind="ExternalInput")
    b2_in = nc.dram_tensor("b2t", [1, 1], dt.float32, kind="ExternalInput")
    outp = nc.dram_tensor("out", [PCH * P, 1], dt.float32, kind="ExternalOutput")

    g_shard = nc.dram_tensor("g_shard", [SHARD, H], dt.float16)
    g_full = nc.dram_tensor("g_full", [NPAD, H], dt.float16, addr_space="Shared")
    # local e-table with 256B row stride; only the first H columns are
    # written/read (the tail pads rows to dma_gather's 256B granularity)
    e_tab = nc.dram_tensor("e_tab", [ESHARD, 2 * H], dt.float16)

    g_pairs = g_full[:, :].rearrange("(r two) f -> r (two f)", two=2)

    with tile.TileContext(nc) as tc:
        nc.gpsimd.load_library(library_config.mlp)

        with (
            tc.tile_pool(name="const", bufs=1) as cpool,
            tc.tile_pool(name="dinvp", bufs=1) as dpool,
        ):
            wc_sb = cpool.tile([P, 2, H], dt.float16)
            nc.sync.dma_start(wc_sb[:], wc_in[:, :, :])
            iota = cpool.tile([P, P], dt.float16)
            nc.sync.dma_start(iota[:], iota_in[:, :])
            ident = cpool.tile([P, P], dt.float16)
            nc.sync.dma_start(ident[:], ident_in[:, :])
            w1_sb = cpool.tile([P, 16], dt.float16)
            nc.sync.dma_start(w1_sb[:], w1_in[:, :])
            b1_sb = cpool.tile([16, 1], dt.float32)
            nc.sync.dma_start(b1_sb[:], b1_in[:, :])
            w2_sb = cpool.tile([16, 1], dt.float32)
            nc.sync.dma_start(w2_sb[:], w2_in[:, :])
            b2_sb = cpool.tile([1, 1], dt.float32)
            nc.sync.dma_start(b2_sb[:], b2_in[:, :])
            elocmm_sb = cpool.tile([P, esched["n_mm"]], dt.float16)
            nc.sync.dma_start(elocmm_sb[:], elocmm_in[:, :])

            dinve = dpool.tile([P, NT_E], dt.float32)
            nc.sync.dma_start(dinve[:], dinve_in[:, :])
            bde_sb = dpool.tile([P, NT_E, H], dt.float16)
            nc.sync.dma_start(bde_sb[:], bde_in[:, :, :])

            # phase-D prep hoisted to the head: pair index tiles, loc planes
            # and one-hot builds depend only on inputs, so they run while
            # phase A / the AllGather init own the other engines
            NCH_P = psched["totchunks"]
            plocmm_sb = dpool.tile([P, psched["n_mm"]], dt.float16)
            nc.scalar.dma_start(plocmm_sb[:], plocmm_in[:, :])
            pidxt = dpool.tile([P, psched["totidx"] // 16], dt.int16)
            nc.scalar.dma_start(pidxt[:], pgidx_in[:, :])
            poh_sb = dpool.tile([P, NCH_P, P], dt.float16)
            nc.vector.tensor_tensor(
                poh_sb[:],
                plocmm_sb[:, 0:NCH_P].unsqueeze(2).to_broadcast([P, NCH_P, P]),
                iota[:, :].unsqueeze(1).to_broadcast([P, NCH_P, P]),
                mybir.AluOpType.is_equal,
            )


            def _one_pass():
                # ------- phase A: g = x' @ W, AllGather split in halves -------
                XBLK = 49
                HTILES = HALF // P        # tiles 0..48 cover rows [0, HALF)
                assert HTILES % XBLK == 0
                with (
                    tc.tile_pool(name="xtp", bufs=2) as xtp,
                    tc.tile_pool(name="hps", bufs=4, space="PSUM") as hps,
                    tc.tile_pool(name="gsb", bufs=1) as gsbp,
                ):
                    g_sb = gsbp.tile([P, TILES, H], dt.float16)
                    for blk in range((TILES + XBLK - 1) // XBLK):
                        t0, t1 = blk * XBLK, min((blk + 1) * XBLK, TILES)
                        xt_sb = xtp.tile([P, 2, (t1 - t0) * P], dt.float16, tag="xt")
                        nc.sync.dma_start(xt_sb[:], xt_in[:, :, t0 * P: t1 * P])
                        for t in range(t0, t1):
                            h_ps = hps.tile([P, H], dt.float32)
                            for k in range(2):
                                nc.tensor.matmul(
                                    h_ps[:],
                                    lhsT=xt_sb[:, k, (t - t0) * P:(t - t0 + 1) * P],
                                    rhs=wc_sb[:, k, :],
                                    start=(k == 0), stop=(k == 1),
                                )
                            nc.scalar.activation(
                                g_sb[:, t, :], h_ps[:],
                                mybir.ActivationFunctionType.Copy,
                                bias=0.0, scale=1.0)
                        if t1 == HTILES:
                            # first half ready: store + gather while 2nd half runs
                            nc.sync.dma_start(
                                g_shard[0:HALF, :].rearrange(
                                    "(t p) f -> p t f", p=P),
                                g_sb[:, 0:HALF // P, :],
                            )
                            nc.gpsimd.collective_compute(
                                "AllGather", mybir.AluOpType.bypass,
                                replica_groups=[list(range(NC))],
                                ins=[g_shard[0:HALF, :].opt()],
                                outs=[g_full[0:NC * HALF, :].opt()],
                            )
                    nc.sync.dma_start(
                        g_shard[HALF:, :].rearrange("(t p) f -> p t f", p=P),
                        g_sb[:, HALF // P:, :],
                    )

                nc.gpsimd.collective_compute(
                    "AllGather", mybir.AluOpType.bypass,
                    replica_groups=[list(range(NC))],
                    ins=[g_shard[HALF:, :].opt()],
                    outs=[g_full[NC * HALF:, :].opt()],
                )

                # ---------------- phase C: aggregate per dst tile ----------------
                with (
                    tc.tile_pool(name="ewidx", bufs=3) as widxp,
                    tc.tile_pool(name="emsg", bufs=3) as msgp,
                    tc.tile_pool(name="eoh", bufs=3) as ohp,
                    tc.tile_pool(name="eacc", bufs=2, space="PSUM") as accp,
                    tc.tile_pool(name="eemb", bufs=4) as embp,
                ):
                    def acc_init_edge(t, a):
                        # seed PSUM with bconv/dinv so consume is one Lrelu
                        nc.tensor.matmul(a[:], lhsT=ident[:],
                                         rhs=bde_sb[:, t, :],
                                         start=True, stop=False)

                    def consume_edge(t, a):
                        emb = embp.tile([P, H], dt.float16, name=f"emb_{t}")
                        nc.scalar.activation(
                            emb[:], a[:], mybir.ActivationFunctionType.Lrelu,
                            bias=0.0, scale=dinve[:, t:t + 1], alpha=NEG)
                        nc.sync.dma_start(e_tab[t * P:(t + 1) * P, 0:H], emb[:])

                    e_src_aps = [g_pairs, g_pairs,
                                 g_pairs[HR:, :], g_pairs[HR:, :]]
                    _emit_scatter2(nc, dt, e_src_aps, egidx_in, elocmm_sb, iota,
                                   esched, (widxp, msgp, ohp, accp),
                                   consume_edge, "e", acc_init=acc_init_edge)

                # ------- phase D: pair gather + permute-matmul + MLP -------
                # pair slots are laid out so gather chunk == slot tile and
                # tiles (2k, 2k+1) hold pair-chunk k's xi/xj lookups; one
                # matmul msg.T @ onehot produces each 64-row half of xijt
                # (feature-major, pair-ordered) straight from the gather.
                # MLP z/o stages run 4 pair-chunks wide (N=512).
                with (
                    tc.tile_pool(name="pmsg", bufs=2) as pmsgp,
                    tc.tile_pool(name="ptps", bufs=3, space="PSUM") as ptps,
                    tc.tile_pool(name="pzps", bufs=2, space="PSUM") as pzps,
                    tc.tile_pool(name="pops", bufs=2, space="PSUM") as pops,
                    tc.tile_pool(name="psb", bufs=3) as psbp,
                ):
                    nch_w = GROUP_P
                    nidx_w = nch_w * P
                    msg_w = {}
                    for g in range(psched["ngroups"]):
                        msg = pmsgp.tile([P, nch_w, P], dt.float16,
                                         tag="pmsg", name=f"pmsg{g}")
                        nc.gpsimd.dma_gather(
                            msg[:], e_tab[:, :],
                            pidxt[:, g * nidx_w // 16:(g + 1) * nidx_w // 16],
                            nidx_w, nidx_w, P,
                            single_packet=False, queue_num=0)
                        msg_w[g] = msg

                    KB = 4
                    for kb in range(PCH // KB):
                        xw = psbp.tile([P, KB * P], dt.float16, tag="xw")
                        for kk in range(KB):
                            k = kb * KB + kk
                            xt_ps = ptps.tile([P, P], dt.float32)
                            for half in range(2):
                                t = 2 * k + half
                                nc.tensor.matmul(
                                    xt_ps[half * H:(half + 1) * H, :],
                                    lhsT=msg_w[t // GROUP_P][:, t % GROUP_P, 0:H],
                                    rhs=poh_sb[:, t, :],
                                    start=True, stop=True,
                                )
                            nc.vector.tensor_copy(
                                xw[:, kk * P:(kk + 1) * P], xt_ps[:])
                        z_ps = pzps.tile([16, KB * P], dt.float32)
                        nc.tensor.matmul(z_ps[:], lhsT=w1_sb[:], rhs=xw[:],
                                         start=True, stop=True)
                        z2 = psbp.tile([16, KB * P], dt.float32, tag="z2")
                        nc.scalar.activation(
                            z2[:], z_ps[:], mybir.ActivationFunctionType.Lrelu,
                            bias=b1_sb[:, 0:1], scale=1.0, alpha=NEG)
                        o_ps = pops.tile([1, KB * P], dt.float32)
                        nc.tensor.matmul(o_ps[:], lhsT=w2_sb[:], rhs=z2[:],
                                         start=True, stop=True)
                        osb = psbp.tile([1, KB * P], dt.float32, tag="osb")
                        nc.scalar.activation(
                            osb[:], o_ps[:], mybir.ActivationFunctionType.Sigmoid,
                            bias=b2_sb[:, 0:1], scale=1.0)
                        nc.sync.dma_start(
                            outp[kb * KB * P:(kb + 1) * KB * P, :].rearrange(
                                "r one -> one r"),
                            osb[0:1, :])

            for _ in range(passes):
                _one_pass()

    # align each gather's SWDGE queue with its Tile-assigned DMA lane so
    # semaphore<->queue locking stays consistent (4-way parallel desc gen)
    for blk in nc.m.functions[0].blocks:
        for inst in blk.instructions:
            if isinstance(inst, mybir.InstDMAGatherAnt):
                si = inst.sync_info
                for u in (si.on_update if si else []):
                    mm = re.match(r"DMASW(\d+)_", u.ant_name or "")
                    if mm:
                        inst.queue_num = int(mm.group(1)) % 4
                        break

    nc.compile()
    return nc


def kernel(**inputs) -> np.ndarray:
    in_maps, sched = _prep(inputs)
    nc = _build(sched)
    res = run_bass_kernel_spmd(nc, in_maps, list(range(NC)))
    out = np.concatenate([res.results[c]["out"] for c in range(NC)], axis=0)
    full = np.empty_like(out)
    full[sched["pair_perm"]] = out
    return full.astype(np.float32)


# revision 37
# speedup vs baseline: 1.1154x; 1.0111x over previous
"""GCN message-passing kernel for 8 Trainium2 NeuronCores (Bass/Tile).

Computes (matching the jax reference):
    h = x @ W_conv                      [N, H]
    node_embed = leaky_relu(D^-1/2 (A+I) D^-1/2 h + b_conv)
    out = sigmoid(leaky(cat(e[i], e[j]) @ W1 + b1) @ W2 + b2)

Only nodes referenced by `index` (the pair head) contribute to the output,
and each core aggregates exactly the nodes its own pair block references
(~3.8K nodes, ~65K edges per core) — no second AllGather is needed; the
pair head reads the core-local embedding table. Sources come from an
AllGather (split in two, overlapped with phase A) of g = (dinv*x) @ W_conv
with dinv folded into x on the host. Edges are gathered in bulk with
dma_gather (pair-packed fp16 rows) and scatter-added on the TensorEngine
via one-hot matmuls; chunks may straddle destination tiles (one matmul per
(chunk, tile) pair) to minimize index padding, since Q7 descriptor
generation is the pacing resource.
"""

import re

import numpy as np

import concourse.bass as bass
import concourse.bacc as bacc
import concourse.mybir as mybir
import concourse.tile as tile
from concourse import library_config
from concourse.bass_utils import run_bass_kernel_spmd

NC = 8
N_NODES = 100000
F_IN = 256
H = 64
NEG = 0.01

P = 128                    # partitions / tile height
TILES = 98                 # full-node tiles per core (phase A)
SHARD = TILES * P          # 12544 nodes per core
HALF = SHARD // 2          # 6272: phase A / AllGather split granularity
NPAD = NC * SHARD          # 100352
HR = NPAD // 4             # 25088: int16-addressable pair rows per range

GROUP_E = 4                # dst tiles per edge gather group
GROUP_P = 16               # pair slot-tiles per gather window
NOMATCH = -999.0


def _node_remap(n):
    """Original node id -> row in the half-split AllGather layout."""
    c, r = n // SHARD, n % SHARD
    h = r // HALF
    return h * (NC * HALF) + c * HALF + (r - h * HALF)


def _wrap_idx(idx):
    """int array [W] (W % 16 == 0) -> [128, W//16] int16 wrapped/replicated."""
    w = idx.reshape(-1, 16).T.astype(np.int16)
    return np.tile(w, (8, 1))


def _sched2(core, tl, loc, bucket, pidx, ntiles, group_sz, nbucket):
    """Multi-tile-chunk gather/scatter schedule, uniform across cores.

    Each item is gathered from pair-packed row `pidx` (bucket selects the
    source window / rhs parity) and scatter-added to column `loc` of tile
    `tl` on core `core`. Rows are laid out per (core, group, bucket) sorted
    by (tile, pidx); chunks are consecutive 128-row windows; a chunk gets
    one matmul per tile present in ANY core's chunk (union), with per-core
    loc planes masking non-members.

    Returns (sched, idx_i16 [NC,128,totidx//16], locmm_f16 [NC,128,n_mm]).
    """
    ngroups = (ntiles + group_sz - 1) // group_sz
    grp = tl // group_sz

    wid = (core * ngroups + grp) * nbucket + bucket
    n_win = NC * ngroups * nbucket
    cnt = np.bincount(wid, minlength=n_win).reshape(NC, ngroups, nbucket)
    K = (cnt + P - 1) // P
    K = K.max(axis=0)                       # [ngroups, nbucket]

    order = np.lexsort((pidx, tl, bucket, grp, core))
    so_core, so_grp, so_b = core[order], grp[order], bucket[order]
    so_tl, so_loc, so_pidx = tl[order], loc[order], pidx[order]
    so_wid = wid[order]
    starts = np.r_[0, np.flatnonzero(np.diff(so_wid)) + 1]
    run_ids = np.zeros(len(so_wid), np.int64)
    run_ids[starts[1:]] = 1
    run_ids = np.cumsum(run_ids)
    rank = np.arange(len(so_wid)) - starts[run_ids]

    win_c0 = np.zeros((ngroups, nbucket), np.int64)
    acc = 0
    for g in range(ngroups):
        for b in range(nbucket):
            win_c0[g, b] = acc
            acc += K[g, b]
    totchunks = acc
    totidx = totchunks * P

    slot = win_c0[so_grp, so_b] * P + rank
    chunk_of = slot // P

    loc_arr = np.full((NC, totidx), -1, np.int64)
    tl_arr = np.full((NC, totidx), -1, np.int64)
    pidx_arr = np.zeros((NC, totidx), np.int64)
    loc_arr[so_core, slot] = so_loc
    tl_arr[so_core, slot] = so_tl
    pidx_arr[so_core, slot] = so_pidx

    pres = set(zip(chunk_of.tolist(), so_tl.tolist()))
    win_of_chunk = np.zeros(totchunks, np.int64)
    for g in range(ngroups):
        for b in range(nbucket):
            win_of_chunk[win_c0[g, b]: win_c0[g, b] + K[g, b]] = g * nbucket + b

    tiles_with_mm = {t for (_, t) in pres}
    for t in range(ntiles):
        if t not in tiles_with_mm:
            g = t // group_sz
            if K[g, 0] == 0:
                raise RuntimeError("empty window for dummy mm")
            pres.add((int(win_c0[g, 0]), t))

    mms = sorted(pres)
    n_mm = len(mms)

    first_mm = {}
    last_mm = {}
    for j, (c, t) in enumerate(mms):
        first_mm.setdefault(t, j)
        last_mm[t] = j

    win_mms = [[] for _ in range(ngroups * nbucket)]
    for j, (c, t) in enumerate(mms):
        win_mms[win_of_chunk[c]].append((j, c, t))

    locmm = np.full((NC, P, n_mm), NOMATCH, np.float16)
    for cc in range(NC):
        la = loc_arr[cc].reshape(totchunks, P)
        ta = tl_arr[cc].reshape(totchunks, P)
        for j, (c, t) in enumerate(mms):
            m = ta[c] == t
            if m.any():
                locmm[cc, m, j] = la[c][m].astype(np.float16)

    idx_i16 = np.zeros((NC, P, totidx // 16), np.int16)
    for cc in range(NC):
        idx_i16[cc] = _wrap_idx(pidx_arr[cc])

    sched = {
        "ngroups": ngroups,
        "nbucket": nbucket,
        "K": K,
        "win_c0": win_c0,
        "win_mms": win_mms,
        "first_mm": first_mm,
        "last_mm": last_mm,
        "n_mm": n_mm,
        "totchunks": totchunks,
        "totidx": totidx,
        "ntiles": ntiles,
        "group_sz": group_sz,
    }
    return sched, idx_i16, locmm


def _prep(inputs):
    x = np.asarray(inputs["x"], np.float32)
    edge_index = np.asarray(inputs["edge_index"], np.int64)
    index = np.asarray(inputs["index"], np.int64)
    W_conv = np.asarray(inputs["W_conv"], np.float32)
    b_conv = np.asarray(inputs["b_conv"], np.float32)
    W1 = np.asarray(inputs["W1"], np.float32)
    b1 = np.asarray(inputs["b1"], np.float32)
    W2 = np.asarray(inputs["W2"], np.float32)
    b2 = np.asarray(inputs["b2"], np.float32)

    n = x.shape[0]
    src = edge_index[0].astype(np.int64)
    dst = edge_index[1].astype(np.int64)

    deg = np.bincount(dst, minlength=NPAD).astype(np.float32) + 1.0
    deg[n:] = 1.0
    dinv = 1.0 / np.sqrt(deg)

    B = index.shape[0]
    PB = B // NC
    assert PB % P == 0
    PCH = PB // P

    # cluster pairs sharing nodes onto the same core (connected components +
    # greedy packing): ~13% fewer replicated aggregation slots. The final
    # output is un-permuted on the host.
    import scipy.sparse as _sp
    from scipy.sparse.csgraph import connected_components as _cc
    pp = np.arange(B)
    rows = np.concatenate([pp, pp])
    cols = np.concatenate([B + index[:, 0], B + index[:, 1]])
    adj = _sp.coo_matrix(
        (np.ones(2 * B, np.int8), (rows, cols)), shape=(B + n, B + n))
    _, labels = _cc(adj, directed=False)
    roots = labels[:B]
    order0 = np.argsort(roots, kind="stable")
    runs = np.r_[0, np.flatnonzero(np.diff(roots[order0])) + 1, B]
    comps = [order0[runs[i]:runs[i + 1]] for i in range(len(runs) - 1)]
    comps.sort(key=len, reverse=True)
    bins = [[] for _ in range(NC)]
    loads = [0] * NC
    for plist in comps:
        cands = [c for c in range(NC) if loads[c] + len(plist) <= PB]
        if cands:
            c = min(cands, key=lambda c: loads[c])
            bins[c].extend(plist.tolist())
            loads[c] += len(plist)
        else:
            for p in plist.tolist():
                c = min(range(NC), key=lambda c: loads[c])
                bins[c].append(p)
                loads[c] += 1
    assert loads == [PB] * NC
    pair_perm = np.arange(B, dtype=np.int64)   # clustering disabled

    # per-core needed node sets (each core aggregates what its pairs read)
    uniq_c = [np.unique(index[c * PB:(c + 1) * PB]) for c in range(NC)]
    NT_E = max(-(-len(u) // P) for u in uniq_c)
    ESHARD = NT_E * P
    slot_of = np.full((NC, n), -1, np.int64)
    for c in range(NC):
        slot_of[c, uniq_c[c]] = np.arange(len(uniq_c[c]))

    # edge stream: an edge goes to every core that needs its dst
    src_new = _node_remap(src)
    uniq_new = [_node_remap(u) for u in uniq_c]
    e_core, e_node, e_slot = [], [], []
    for c in range(NC):
        keep = slot_of[c, dst] >= 0
        e_node.append(src_new[keep])
        e_slot.append(slot_of[c, dst[keep]])
        e_core.append(np.full(keep.sum(), c, np.int64))
        # self-loops
        e_node.append(uniq_new[c])
        e_slot.append(np.arange(len(uniq_c[c]), dtype=np.int64))
        e_core.append(np.full(len(uniq_c[c]), c, np.int64))
    es = np.concatenate(e_node)
    ed = np.concatenate(e_slot)
    ec = np.concatenate(e_core)

    ebucket = 2 * (es >= 2 * HR).astype(np.int64) + (es & 1)
    epidx = (es >> 1) - (ebucket >= 2) * HR
    esched, eidx, elocmm = _sched2(
        core=ec, tl=ed // P, loc=ed % P,
        bucket=ebucket, pidx=epidx, ntiles=NT_E, group_sz=GROUP_E, nbucket=4)

    # pair stream over each core's local e-table (256B-stride rows, so the
    # slot IS the gather index: no parity buckets, zero padding); slots
    # interleaved so pair chunk k reads slot-tiles (2k, 2k+1) -> MLP
    # pipelines with the gather stream
    pair_global = np.arange(B, dtype=np.int64)
    pcore = pair_global // PB
    plocal = pair_global % PB
    s_core = np.concatenate([pcore, pcore])
    ch = plocal // P
    col = plocal % P
    s_slot = np.concatenate([(2 * ch) * P + col, (2 * ch + 1) * P + col])
    s_node = np.concatenate([
        slot_of[pcore, index[:, 0]], slot_of[pcore, index[:, 1]]])
    assert (s_node >= 0).all()
    psched, pidx, plocmm = _sched2(
        core=s_core, tl=s_slot // P, loc=s_slot % P,
        bucket=np.zeros_like(s_node), pidx=s_node,
        ntiles=2 * PCH, group_sz=GROUP_P, nbucket=1)

    # host-folded dinv: g = (dinv * x) @ W
    xpad = np.zeros((NPAD, F_IN), np.float32)
    xpad[:n] = x * dinv[:n, None]
    xT = xpad.T.astype(np.float16)
    xT_shards = [
        np.ascontiguousarray(
            xT[:, c * SHARD:(c + 1) * SHARD].reshape(2, P, SHARD).transpose(1, 0, 2)
        ) for c in range(NC)
    ]
    # dinv over each core's local e-slot space, and bconv/dinv bias tables
    # (added into the PSUM accumulator via an identity matmul so the whole
    # consume is one Lrelu activation)
    dinv_e_sb = []
    bde_sb = []
    for c in range(NC):
        d = np.zeros(ESHARD, np.float32)
        d[:len(uniq_c[c])] = dinv[uniq_c[c]]
        dinv_e_sb.append(np.ascontiguousarray(d.reshape(NT_E, P).T))
        bde = np.zeros((ESHARD, H), np.float32)
        nu = len(uniq_c[c])
        bde[:nu] = b_conv[None, :] / d[:nu, None]
        bde_sb.append(np.ascontiguousarray(
            bde.reshape(NT_E, P, H).transpose(1, 0, 2)).astype(np.float16))

    consts = {
        "wc": np.ascontiguousarray(
            W_conv.reshape(2, P, H).transpose(1, 0, 2)).astype(np.float16),
        "iota": np.broadcast_to(np.arange(P, dtype=np.float16), (P, P)).copy(),
        "ident": np.eye(P, dtype=np.float16),
        "w1": W1.astype(np.float16),
        "b1": b1.reshape(16, 1).astype(np.float32),
        "w2": W2.astype(np.float32),
        "b2t": b2.reshape(1, 1).astype(np.float32),
    }
    sched = {"edge": esched, "pair": psched, "PCH": PCH,
             "NT_E": NT_E, "ESHARD": ESHARD, "pair_perm": pair_perm}
    in_maps = []
    for c in range(NC):
        m = {
            "xt": xT_shards[c],
            "dinve": dinv_e_sb[c],
            "bde": bde_sb[c],
            "elocmm": elocmm[c],
            "egidx": eidx[c],
            "plocmm": plocmm[c],
            "pgidx": pidx[c],
        }
        m.update(consts)
        in_maps.append(m)
    return in_maps, sched


def _emit_scatter2(nc, dt, src_aps, idx_dram, locmm_sb, iota, sched,
                   pools, consume, prefix, post_group=None, idx_eng=None,
                   acc_init=None):
    """Gather pair-packed rows per window, build per-matmul one-hot planes,
    matmul-accumulate into per-tile PSUM, hand finished tiles to consume.

    src_aps: bucket -> source AP (pair-packed rows).
    """
    widxp, msgp, ohp, accp = pools
    if idx_eng is None:
        idx_eng = nc.sync
    ngroups = sched["ngroups"]
    nbucket = sched["nbucket"]
    K = sched["K"]
    win_c0 = sched["win_c0"]
    win_mms = sched["win_mms"]
    first_mm = sched["first_mm"]
    last_mm = sched["last_mm"]
    group_sz = sched["group_sz"]
    ntiles = sched["ntiles"]

    acc_tiles = {}

    def _drain(g):
        # consume runs one group late so the DVE queue doesn't head-block
        # on PSUM drains while later windows' one-hots are still pending
        for t in range(g * group_sz, min((g + 1) * group_sz, ntiles)):
            consume(t, acc_tiles.pop(t))
        if post_group is not None:
            post_group(g)

    for g in range(ngroups):
        for b in range(nbucket):
            nch = int(K[g, b])
            if nch == 0:
                continue
            c0 = int(win_c0[g, b])
            nidx = nch * P
            idxt = widxp.tile([P, nidx // 16], dt.int16, tag=f"{prefix}idx{b}",
                              name=f"{prefix}idx_g{g}b{b}")
            idx_eng.dma_start(
                idxt[:], idx_dram[:, c0 * P // 16:(c0 + nch) * P // 16])
            msg = msgp.tile([P, nch, P], dt.float16, tag=f"{prefix}msg{b}",
                            name=f"{prefix}msg_g{g}b{b}")
            nc.gpsimd.dma_gather(
                msg[:], src_aps[b], idxt[:], nidx, nidx, P,
                single_packet=False, queue_num=0)
            mms = win_mms[g * nbucket + b]
            if not mms:
                continue
            j0 = mms[0][0]
            n_mm_w = len(mms)
            oh = ohp.tile([P, n_mm_w, P], dt.float16, tag=f"{prefix}oh{b}",
                          name=f"{prefix}oh_g{g}b{b}")
            nc.vector.tensor_tensor(
                oh[:],
                locmm_sb[:, j0:j0 + n_mm_w].unsqueeze(2).to_broadcast(
                    [P, n_mm_w, P]),
                iota[:, :].unsqueeze(1).to_broadcast([P, n_mm_w, P]),
                mybir.AluOpType.is_equal,
            )
            par = b & 1
            for (j, c, t) in mms:
                if t not in acc_tiles:
                    acc_tiles[t] = accp.tile(
                        [P, H], dt.float32, tag=f"{prefix}acc{t % group_sz}",
                        name=f"{prefix}acc_t{t}")
                    if acc_init is not None:
                        acc_init(t, acc_tiles[t])
                nc.tensor.matmul(
                    acc_tiles[t][:],
                    lhsT=oh[:, j - j0, :],
                    rhs=msg[:, c - c0, par * H:(par + 1) * H],
                    start=(acc_init is None and j == first_mm[t]),
                    stop=(j == last_mm[t]),
                )
        if g > 0:
            _drain(g - 1)
    _drain(ngroups - 1)


def _build(sched, passes=1):
    dt = mybir.dt
    esched = sched["edge"]
    psched = sched["pair"]
    PCH = sched["PCH"]
    NT_E = sched["NT_E"]
    ESHARD = sched["ESHARD"]

    nc = bacc.Bacc("TRN2", target_bir_lowering=False, debug=False,
                   enable_asserts=False, num_devices=NC, num_swdge_queues=4)

    xt_in = nc.dram_tensor("xt", [P, 2, SHARD], dt.float16, kind="ExternalInput")
    dinve_in = nc.dram_tensor("dinve", [P, NT_E], dt.float32,
                              kind="ExternalInput")
    bde_in = nc.dram_tensor("bde", [P, NT_E, H], dt.float16,
                            kind="ExternalInput")
    elocmm_in = nc.dram_tensor("elocmm", [P, esched["n_mm"]], dt.float16,
                               kind="ExternalInput")
    egidx_in = nc.dram_tensor("egidx", [P, esched["totidx"] // 16], dt.int16,
                              kind="ExternalInput")
    plocmm_in = nc.dram_tensor("plocmm", [P, psched["n_mm"]], dt.float16,
                               kind="ExternalInput")
    pgidx_in = nc.dram_tensor("pgidx", [P, psched["totidx"] // 16], dt.int16,
                              kind="ExternalInput")
    wc_in = nc.dram_tensor("wc", [P, 2, H], dt.float16, kind="ExternalInput")
    iota_in = nc.dram_tensor("iota", [P, P], dt.float16, kind="ExternalInput")
    ident_in = nc.dram_tensor("ident", [P, P], dt.float16, kind="ExternalInput")
    w1_in = nc.dram_tensor("w1", [P, 16], dt.float16, kind="ExternalInput")
    b1_in = nc.dram_tensor("b1", [16, 1], dt.float32, kind="ExternalInput")
    w2_in = nc.dram_tensor("w2", [16, 1], dt.float32, kind="ExternalInput")
    b2_in = nc.dram_tensor("b2t", [1, 1], dt.float32, kind="ExternalInput")
    outp = nc.dram_tensor("out", [PCH * P, 1], dt.float32, kind="ExternalOutput")

    g_shard = nc.dram_tensor("g_shard", [SHARD, H], dt.float16)
    g_full = nc.dram_tensor("g_full", [NPAD, H], dt.float16, addr_space="Shared")
    # local e-table with 256B row stride; only the first H columns are
    # written/read (the tail pads rows to dma_gather's 256B granularity)
    e_tab = nc.dram_tensor("e_tab", [ESHARD, 2 * H], dt.float16)

    g_pairs = g_full[:, :].rearrange("(r two) f -> r (two f)", two=2)

    with tile.TileContext(nc) as tc:
        nc.gpsimd.load_library(library_config.mlp)

        with (
            tc.tile_pool(name="const", bufs=1) as cpool,
            tc.tile_pool(name="dinvp", bufs=1) as dpool,
        ):
            wc_sb = cpool.tile([P, 2, H], dt.float16)
            nc.sync.dma_start(wc_sb[:], wc_in[:, :, :])
            iota = cpool.tile([P, P], dt.float16)
            nc.sync.dma_start(iota[:], iota_in[:, :])
            ident = cpool.tile([P, P], dt.float16)
            nc.sync.dma_start(ident[:], ident_in[:, :])
            w1_sb = cpool.tile([P, 16], dt.float16)
            nc.sync.dma_start(w1_sb[:], w1_in[:, :])
            b1_sb = cpool.tile([16, 1], dt.float32)
            nc.sync.dma_start(b1_sb[:], b1_in[:, :])
            w2_sb = cpool.tile([16, 1], dt.float32)
            nc.sync.dma_start(w2_sb[:], w2_in[:, :])
            b2_sb = cpool.tile([1, 1], dt.float32)
            nc.sync.dma_start(b2_sb[:], b2_in[:, :])
            elocmm_sb = cpool.tile([P, esched["n_mm"]], dt.float16)
            nc.sync.dma_start(elocmm_sb[:], elocmm_in[:, :])

            dinve = dpool.tile([P, NT_E], dt.float32)
            nc.sync.dma_start(dinve[:], dinve_in[:, :])
            bde_sb = dpool.tile([P, NT_E, H], dt.float16)
            nc.sync.dma_start(bde_sb[:], bde_in[:, :, :])

            # phase-D prep hoisted to the head: pair index tiles, loc planes
            # and one-hot builds depend only on inputs, so they run while
            # phase A / the AllGather init own the other engines
            NCH_P = psched["totchunks"]
            plocmm_sb = dpool.tile([P, psched["n_mm"]], dt.float16)
            nc.scalar.dma_start(plocmm_sb[:], plocmm_in[:, :])
            pidxt = dpool.tile([P, psched["totidx"] // 16], dt.int16)
            nc.scalar.dma_start(pidxt[:], pgidx_in[:, :])
            poh_sb = dpool.tile([P, NCH_P, P], dt.float16)
            nc.vector.tensor_tensor(
                poh_sb[:],
                plocmm_sb[:, 0:NCH_P].unsqueeze(2).to_broadcast([P, NCH_P, P]),
                iota[:, :].unsqueeze(1).to_broadcast([P, NCH_P, P]),
                mybir.AluOpType.is_equal,
            )


            def _one_pass():
                # ------- phase A: g = x' @ W, AllGather split in halves -------
                XBLK = 49
                HTILES = HALF // P        # tiles 0..48 cover rows [0, HALF)
                assert HTILES % XBLK == 0
                with (
                    tc.tile_pool(name="xtp", bufs=2) as xtp,
                    tc.tile_pool(name="hps", bufs=4, space="PSUM") as hps,
                    tc.tile_pool(name="gsb", bufs=1) as gsbp,
                ):
                    g_sb = gsbp.tile([P, TILES, H], dt.float16)
                    for blk in range((TILES + XBLK - 1) // XBLK):
                        t0, t1 = blk * XBLK, min((blk + 1) * XBLK, TILES)
                        xt_sb = xtp.tile([P, 2, (t1 - t0) * P], dt.float16, tag="xt")
                        nc.sync.dma_start(xt_sb[:], xt_in[:, :, t0 * P: t1 * P])
                        for t in range(t0, t1):
                            h_ps = hps.tile([P, H], dt.float32)
                            for k in range(2):
                                nc.tensor.matmul(
                                    h_ps[:],
                                    lhsT=xt_sb[:, k, (t - t0) * P:(t - t0 + 1) * P],
                                    rhs=wc_sb[:, k, :],
                                    start=(k == 0), stop=(k == 1),
                                )
                            nc.scalar.activation(
                                g_sb[:, t, :], h_ps[:],
                                mybir.ActivationFunctionType.Copy,
                                bias=0.0, scale=1.0)
                        if t1 == HTILES:
                            # first half ready: store + gather while 2nd half runs
                            nc.sync.dma_start(
                                g_shard[0:HALF, :].rearrange(
                                    "(t p) f -> p t f", p=P),
                                g_sb[:, 0:HALF // P, :],
                            )
                            nc.gpsimd.collective_compute(
                                "AllGather", mybir.AluOpType.bypass,
                                replica_groups=[list(range(NC))],
                                ins=[g_shard[0:HALF, :].opt()],
                                outs=[g_full[0:NC * HALF, :].opt()],
                            )
                    nc.sync.dma_start(
                        g_shard[HALF:, :].rearrange("(t p) f -> p t f", p=P),
                        g_sb[:, HALF // P:, :],
                    )

                nc.gpsimd.collective_compute(
                    "AllGather", mybir.AluOpType.bypass,
                    replica_groups=[list(range(NC))],
                    ins=[g_shard[HALF:, :].opt()],
                    outs=[g_full[NC * HALF:, :].opt()],
                )

                # ---------------- phase C: aggregate per dst tile ----------------
                with (
                    tc.tile_pool(name="ewidx", bufs=3) as widxp,
                    tc.tile_pool(name="emsg", bufs=2) as msgp,
                    tc.tile_pool(name="eoh", bufs=4) as ohp,
                    tc.tile_pool(name="eacc", bufs=2, space="PSUM") as accp,
                    tc.tile_pool(name="eemb", bufs=4) as embp,
                ):
                    def acc_init_edge(t, a):
                        # seed PSUM with bconv/dinv so consume is one Lrelu
                        nc.tensor.matmul(a[:], lhsT=ident[:],
                                         rhs=bde_sb[:, t, :],
                                         start=True, stop=False)

                    def consume_edge(t, a):
                        emb = embp.tile([P, H], dt.float16, name=f"emb_{t}")
                        nc.scalar.activation(
                            emb[:], a[:], mybir.ActivationFunctionType.Lrelu,
                            bias=0.0, scale=dinve[:, t:t + 1], alpha=NEG)
                        nc.sync.dma_start(e_tab[t * P:(t + 1) * P, 0:H], emb[:])

                    e_src_aps = [g_pairs, g_pairs,
                                 g_pairs[HR:, :], g_pairs[HR:, :]]
                    _emit_scatter2(nc, dt, e_src_aps, egidx_in, elocmm_sb, iota,
                                   esched, (widxp, msgp, ohp, accp),
                                   consume_edge, "e", acc_init=acc_init_edge,
                                   idx_eng=nc.scalar)

                # ------- phase D: pair gather + permute-matmul + MLP -------
                # pair slots are laid out so gather chunk == slot tile and
                # tiles (2k, 2k+1) hold pair-chunk k's xi/xj lookups; one
                # matmul msg.T @ onehot produces each 64-row half of xijt
                # (feature-major, pair-ordered) straight from the gather.
                # MLP z/o stages run 4 pair-chunks wide (N=512).
                with (
                    tc.tile_pool(name="pmsg", bufs=2) as pmsgp,
                    tc.tile_pool(name="ptps", bufs=3, space="PSUM") as ptps,
                    tc.tile_pool(name="pzps", bufs=2, space="PSUM") as pzps,
                    tc.tile_pool(name="pops", bufs=2, space="PSUM") as pops,
                    tc.tile_pool(name="psb", bufs=3) as psbp,
                ):
                    nch_w = GROUP_P
                    nidx_w = nch_w * P
                    msg_w = {}
                    for g in range(psched["ngroups"]):
                        msg = pmsgp.tile([P, nch_w, P], dt.float16,
                                         tag="pmsg", name=f"pmsg{g}")
                        nc.gpsimd.dma_gather(
                            msg[:], e_tab[:, :],
                            pidxt[:, g * nidx_w // 16:(g + 1) * nidx_w // 16],
                            nidx_w, nidx_w, P,
                            single_packet=False, queue_num=0)
                        msg_w[g] = msg

                    KB = 4
                    for kb in range(PCH // KB):
                        xw = psbp.tile([P, KB * P], dt.float16, tag="xw")
                        for kk in range(KB):
                            k = kb * KB + kk
                            xt_ps = ptps.tile([P, P], dt.float32)
                            for half in range(2):
                                t = 2 * k + half
                                nc.tensor.matmul(
                                    xt_ps[half * H:(half + 1) * H, :],
                                    lhsT=msg_w[t // GROUP_P][:, t % GROUP_P, 0:H],
                                    rhs=poh_sb[:, t, :],
                                    start=True, stop=True,
                                )
                            nc.vector.tensor_copy(
                                xw[:, kk * P:(kk + 1) * P], xt_ps[:])
                        z_ps = pzps.tile([16, KB * P], dt.float32)
                        nc.tensor.matmul(z_ps[:], lhsT=w1_sb[:], rhs=xw[:],
                                         start=True, stop=True)
                        z2 = psbp.tile([16, KB * P], dt.float32, tag="z2")
                        nc.scalar.activation(
                            z2[:], z_ps[:], mybir.ActivationFunctionType.Lrelu,
                            bias=b1_sb[:, 0:1], scale=1.0, alpha=NEG)
                        o_ps = pops.tile([1, KB * P], dt.float32)
                        nc.tensor.matmul(o_ps[:], lhsT=w2_sb[:], rhs=z2[:],
                                         start=True, stop=True)
                        osb = psbp.tile([1, KB * P], dt.float32, tag="osb")
                        nc.scalar.activation(
                            osb[:], o_ps[:], mybir.ActivationFunctionType.Sigmoid,
                            bias=b2_sb[:, 0:1], scale=1.0)
                        nc.sync.dma_start(
                            outp[kb * KB * P:(kb + 1) * KB * P, :].rearrange(
                                "r one -> one r"),
                            osb[0:1, :])

            for _ in range(passes):
                _one_pass()

    # align each gather's SWDGE queue with its Tile-assigned DMA lane so
    # semaphore<->queue locking stays consistent (4-way parallel desc gen)
    for blk in nc.m.functions[0].blocks:
        for inst in blk.instructions:
            if isinstance(inst, mybir.InstDMAGatherAnt):
                si = inst.sync_info
                for u in (si.on_update if si else []):
                    mm = re.match(r"DMASW(\d+)_", u.ant_name or "")
                    if mm:
                        inst.queue_num = int(mm.group(1)) % 4
                        break

    nc.compile()
    return nc


def kernel(**inputs) -> np.ndarray:
    in_maps, sched = _prep(inputs)
    nc = _build(sched)
    res = run_bass_kernel_spmd(nc, in_maps, list(range(NC)))
    out = np.concatenate([res.results[c]["out"] for c in range(NC)], axis=0)
    full = np.empty_like(out)
    full[sched["pair_perm"]] = out
    return full.astype(np.float32)
